# revision 59
# baseline (speedup 1.0000x reference)
"""Multi-head self-attention (B=4, S=2048, D=1024, H=16, Hd=64) on 8 TRN2 cores.

Sharding: core c -> (batch b = c//2, head-group g = c%2 of 8 heads).
Each core computes its batch's 8 heads end-to-end plus the partial output
projection for its head group; the host sums the two head-group partials
per batch. No collectives.

Device layout is fully transposed: activations are [feature(partitions),
seq(free)]. QKV and output projections run in bf16. The attention part
(scores QK^T and PV) runs in fp8e4m3 with DoubleRow perf mode at 0.5
cycles/row. For scores, the two DoubleRow k-subtiles read the SAME 64-dim
Q/K data via a stride-0 middle dim (computing 2*QK^T); the doubling is
folded into the exp scale (1/16 instead of 1/8). V is stored as
[128 keys x kc-pair x 8 head x 65] with an all-ones 65th column producing
the softmax denominator inside the PV psum.

exp() splits across the Activation engine (exact exp, fp8 output) and the
DVE (Schraudolph: fp8e4m3 bits = trunc(s*scale*8/ln2 + 56.05) via one
tensor_scalar mult+add writing int8). Attention is pipelined as 16 jobs
(head, q-half) with PV lagging 4 jobs; Q/K projection m-tiles 1..3 are
emitted piecewise between score chunks so the PE fills exp-bound gaps.
K's bias is dropped (softmax-invariant); V's bias is folded into the
output-projection bias on the host.
"""

from contextlib import ExitStack

import numpy as np
import ml_dtypes

import concourse.bass as bass
import concourse.tile as tile
from concourse import mybir
from concourse.bass_utils import run_bass_kernel_spmd
from concourse.vector_clock import ScopedClock
from bass_rust import InstNoOp, SyncInfo

BF16 = mybir.dt.bfloat16
F32 = mybir.dt.float32
FP8 = mybir.dt.float8e4
I8 = mybir.dt.int8
AF = mybir.ActivationFunctionType
ALU = mybir.AluOpType
DR = mybir.MatmulPerfMode.DoubleRow

B, S, D = 4, 2048, 1024
H, HD = 16, 64
GH = 8          # heads per core (head-group size)
GM = GH * HD    # 512 head dims per core
NDC = 8         # d chunks of 128 (contraction for projections)
NKC = 16        # k chunks of 128
VW = HD + 1     # 65: per-head V columns + ones column
VWP = HD + 2    # 66: v8 stores heads 2-byte aligned (dual-fp8 LW requires
                # even weight byte offsets; 65-wide heads would give odd ones)
JW = S // 2     # 1024: q-width of one attention job
PV_LAG = 4      # jobs between scores(j) and PV(j)

# scores psum holds 2*QK^T (stride-0 DoubleRow pair), so exp scale is 1/16.
EXP_SCALE = 0.0625
SCH_MUL = EXP_SCALE * 8.0 / float(np.log(2.0))
SCH_ADD = 56.05


def _mk_pattern(counts):
    accs = {k: 0.0 for k in counts}
    total = sum(counts.values())
    out = []
    for _ in range(total):
        for k in counts:
            accs[k] += counts[k] / total
        k = max(accs, key=lambda q: accs[q])
        accs[k] -= 1.0
        out.append(k)
    return out


# exp engine per (kc) chunk within a job: 'A' = Act exact, 'D' = DVE
# Schraudolph. Early jobs carry QK copies on Act + V copies on DVE; later
# jobs carry norm work on DVE.
_PAT_EARLY = _mk_pattern({"A": 8, "D": 8})
_PAT_STEADY = _mk_pattern({"A": 10, "D": 6})
_PAT_STEADY2 = _mk_pattern({"A": 9, "D": 7})

_META_TYPES = ("TileBranchInst", "BassTileLoopBlock", "BassTilePoolBoundary")


class _TileCtx(tile.TileContext):
    """Splits multi-sem-wait instructions: the pinned walrus rejects any TPB
    instruction carrying more than one sem-wait, while Tile emits joins and a
    global end-of-context drain with several."""

    def _split_waits(self, ordered):
        nc = self.nc
        for bb_name, insts in ordered.items():
            out = []
            for inst in insts:
                si = inst.sync_info
                if (
                    si is not None
                    and si.on_wait
                    and len(si.on_wait) > 1
                    and type(inst).__name__ not in _META_TYPES
                    and inst.engine != mybir.EngineType.Unassigned
                ):
                    waits = list(si.on_wait)
                    for w in waits[:-1]:
                        nop = InstNoOp(
                            name=nc.get_next_instruction_name(), ins=[], outs=[]
                        )
                        nop.engine = inst.engine
                        nop.sync_info = SyncInfo(on_wait=[w], on_update=[])
                        out.append(nop)
                    inst.sync_info = SyncInfo(
                        on_wait=[waits[-1]], on_update=list(si.on_update)
                    )
                out.append(inst)
            ordered[bb_name] = out

    def _lower_ordered_insts(self, ordered):
        self._split_waits(ordered)
        super()._lower_ordered_insts(ordered)

    def _drain_and_barrier(self, tick_clock, wait_clock):
        drain_inst = self.nc.sync.drain()
        wait_clock.add_sem_waits(
            drain_inst.ins, ScopedClock({None: tick_clock.global_clock})
        )
        si = drain_inst.ins.sync_info
        waits = list(si.on_wait) if si is not None else []
        if len(waits) > 1:
            drain_inst.ins.sync_info = SyncInfo(
                on_wait=waits[:1], on_update=list(si.on_update)
            )
            for w in waits[1:]:
                extra = self.nc.sync.drain()
                extra.ins.sync_info = SyncInfo(on_wait=[w], on_update=[])

        self.nc.all_engine_barrier()
        assert self.sems is not None
        popped = self.nc._tile_sem_poison_stack.pop()
        assert popped is self._sem_poison
        self.nc.clear_and_free_semaphores(list(self.sems.allocated().values()))
        self.nc.all_engine_barrier()


def _build_program():
    nc = bass.Bass(trn_type="TRN2", debug=False, num_devices=8)

    xT = nc.dram_tensor("xT", [D, S], BF16, kind="ExternalInput").ap()
    wq = nc.dram_tensor("wq", [D, GM], BF16, kind="ExternalInput").ap()
    wk = nc.dram_tensor("wk", [D, GM], BF16, kind="ExternalInput").ap()
    wv = nc.dram_tensor("wv", [D, GM], BF16, kind="ExternalInput").ap()
    # pair-major-reordered Wo.T slice: [128, 4 pairs x 1024]
    wo = nc.dram_tensor("wo", [128, (GM // 128) * D], BF16, kind="ExternalInput").ap()
    bq = nc.dram_tensor("bq", [GM], F32, kind="ExternalInput").ap()
    bo = nc.dram_tensor("bo", [D], F32, kind="ExternalInput").ap()
    outT = nc.dram_tensor("outT", [D, S], F32, kind="ExternalOutput").ap()

    with _TileCtx(nc) as tc, ExitStack() as ctx:
        const_pool = ctx.enter_context(tc.tile_pool(name="const", bufs=1))
        act_pool = ctx.enter_context(tc.tile_pool(name="acts", bufs=1))

        # ---- constants / weights / inputs -------------------------------
        bq_sb = const_pool.tile([128, GM // 128], F32, tag="bq")
        nc.sync.dma_start(bq_sb[:], bq.rearrange("(c p) -> p c", p=128))
        bo_sb = const_pool.tile([128, NDC], F32, tag="bo")
        nc.sync.dma_start(bo_sb[:], bo.rearrange("(c p) -> p c", p=128))
        wo_sb = const_pool.tile([128, (GM // 128) * D], BF16, tag="wo")
        nc.sync.dma_start(wo_sb[:], wo[:, :])

        # persistent activations. qt8/kt8: [128, slot t, S]; head h lives at
        # partitions [64*(h%2), +64) of slot h//2.
        qt8 = act_pool.tile([128, GH // 2, S], FP8, tag="qt8")
        kt8 = act_pool.tile([128, GH // 2, S], FP8, tag="kt8")
        v8 = act_pool.tile([128, NKC // 2, 2, GH, VWP], FP8, tag="v8")
        nc.gpsimd.memset(v8[:, :, :, :, HD:VWP], 1.0)
        otp = [
            act_pool.tile([128, S], BF16, name=f"otp{t}", tag=f"otp{t}")
            for t in range(GH // 2)
        ]

        phase1 = ExitStack()
        w_pool = phase1.enter_context(tc.tile_pool(name="wts", bufs=1))
        xt = w_pool.tile([128, NDC * S], BF16, tag="xt")
        for t in range(NDC):
            eng = nc.sync if t % 2 == 0 else nc.scalar
            eng.dma_start(
                xt[:, t * S : (t + 1) * S], xT[t * 128 : (t + 1) * 128, :]
            )
        wv_sb = w_pool.tile([128, NDC * GM], BF16, tag="wv")
        nc.scalar.dma_start(
            wv_sb[:].rearrange("p (c m) -> p c m", m=GM),
            wv.rearrange("(c p) m -> p c m", p=128),
        )
        wq_sb = w_pool.tile([128, NDC * GM], BF16, tag="wq")
        nc.sync.dma_start(
            wq_sb[:].rearrange("p (c m) -> p c m", m=GM),
            wq.rearrange("(c p) m -> p c m", p=128),
        )
        wk_sb = w_pool.tile([128, NDC * GM], BF16, tag="wk")
        nc.sync.dma_start(
            wk_sb[:].rearrange("p (c m) -> p c m", m=GM),
            wk.rearrange("(c p) m -> p c m", p=128),
        )

        # ---- attention + projections, software-pipelined ----------------
        with tc.tile_pool(name="s_psum", bufs=2, space="PSUM") as s_psum, \
             tc.tile_pool(name="slab", bufs=PV_LAG + 1) as slab_pool, \
             tc.tile_pool(name="norm", bufs=1) as norm_pool, \
             tc.tile_pool(name="bcast", bufs=2) as bc_pool, \
             tc.tile_pool(name="shift", bufs=2) as shift_pool, \
             tc.tile_pool(name="dscr", bufs=2, space="DRAM") as dram_pool:

            def qk_quarter(t, w_sb, dst, qq):
                """Generator: Q or K projection m-tile t, q-quarter qq (512).
                Yields after small matmul groups so callers can interleave."""
                ps = qk_psum.tile([128, 512], F32, tag="qkp")
                q0 = qq * 512
                for dc in range(NDC):
                    nc.tensor.matmul(
                        ps[:],
                        w_sb[:, dc * GM + t * 128 : dc * GM + (t + 1) * 128],
                        xt[:, dc * S + q0 : dc * S + q0 + 512],
                        start=(dc == 0),
                        stop=(dc == NDC - 1),
                    )
                    if dc % 2 == 1:
                        yield
                if dst is qt8:
                    nc.scalar.activation(
                        dst[:, t, q0 : q0 + 512], ps[:], AF.Identity,
                        bias=bq_sb[:, t : t + 1],
                    )
                else:
                    nc.scalar.activation(dst[:, t, q0 : q0 + 512], ps[:], AF.Copy)
                yield

            def v_chunk(si, v_psum):
                """Generator: V projection keys-chunk si -> v8 (fp8)."""
                ps = v_psum.tile([128, GM], F32, tag="vp")
                for dc in range(NDC):
                    nc.tensor.matmul(
                        ps[:],
                        xt[:, dc * S + si * 128 : dc * S + (si + 1) * 128],
                        wv_sb[:, dc * GM : (dc + 1) * GM],
                        start=(dc == 0),
                        stop=(dc == NDC - 1),
                    )
                    if dc % 4 == 3:
                        yield
                nc.vector.tensor_copy(
                    v8[:, si // 2, si % 2, :, 0:HD],
                    ps[:].rearrange("p (h d) -> p h d", h=GH),
                )
                yield

            # jobs: j = 2*h + qh
            slabs = [None] * (2 * GH)

            s_pools = [s_psum]
            chunk_ctr = [0]

            def issue_job_scores(j, fillers, pattern, inject=None):
                """Scores+exp for job j, pulling filler matmul groups from
                `fillers` (a list of active generators) between chunks.
                `inject` maps chunk index -> thunk issued at that point."""
                h, qh = j // 2, j % 2
                slot, p0 = h // 2, 64 * (h % 2)
                sl = slab_pool.tile([128, NKC // 2, 2, JW], FP8, tag="slab")
                slabs[j] = sl
                psl = slice(p0, p0 + 64)
                for kc in range(NKC):
                    if inject and kc in inject:
                        inject[kc]()
                    u, jj = kc // 2, kc % 2
                    # kt8/qt8 [64, 1, N] viewed as a stride-0 [64, 2, N]
                    lhsT = kt8[psl, slot, kc * 128 : (kc + 1) * 128] \
                        .unsqueeze(1).broadcast_to([64, 2, 128])
                    sp = s_pools[chunk_ctr[0] % len(s_pools)].tile(
                        [128, JW], F32, tag="sp"
                    )
                    chunk_ctr[0] += 1
                    for qq in range(JW // 512):
                        q0 = qh * JW + qq * 512
                        rhs = qt8[psl, slot, q0 : q0 + 512] \
                            .unsqueeze(1).broadcast_to([64, 2, 512])
                        nc.tensor.matmul(
                            sp[:, qq * 512 : (qq + 1) * 512],
                            lhsT,
                            rhs,
                            start=True,
                            stop=True,
                            perf_mode=DR,
                        )
                    dst = sl[:, u, jj, :]
                    if pattern[kc] == "A":
                        nc.scalar.activation(dst, sp[:], AF.Exp, scale=EXP_SCALE)
                    else:
                        nc.vector.tensor_scalar(
                            dst.bitcast(I8), sp[:],
                            SCH_MUL, SCH_ADD, ALU.mult, ALU.add,
                        )
                    # pull ~1.4 filler steps per chunk so the 12 QK halves
                    # (slots 1-3) finish by the end of job 4
                    fillers_budget[0] += 1.4
                    while fillers and fillers_budget[0] >= 1.0:
                        try:
                            next(fillers[0])
                            fillers_budget[0] -= 1.0
                        except StopIteration:
                            fillers.pop(0)

            pending = {}  # j -> (po tile, bc tile)

            def issue_pv_den(j, o_psum):
                """PV matmuls + reciprocal + the den-broadcast DMA roundtrip.
                The otp multiply is deferred (issue_norm_mult) so the DVE
                never stalls in-order on the broadcast DMA."""
                h = j // 2
                sl = slabs[j]
                po = o_psum.tile([128, JW], F32, tag="op")
                for u in range(NKC // 2):
                    lhsT = v8[:, u, :, h, 0:VW]
                    for qs in range(JW // 512):
                        nc.tensor.matmul(
                            po[0:VW, qs * 512 : (qs + 1) * 512],
                            lhsT,
                            sl[:, u, :, qs * 512 : (qs + 1) * 512],
                            start=(u == 0),
                            stop=(u == NKC // 2 - 1),
                            perf_mode=DR,
                        )
                nrm = norm_pool.tile([VW, JW], F32, tag="nrm")
                nc.vector.reciprocal(nrm[HD:VW, :], po[HD:VW, :])
                scr = dram_pool.tile([JW], F32, tag="scr")
                nc.sync.dma_start(scr.unsqueeze(0), nrm[HD:VW, :])
                bc = bc_pool.tile([HD, JW], F32, tag="bc")
                nc.sync.dma_start(bc[:], scr.unsqueeze(0).broadcast_to([HD, JW]))
                pending[j] = (po, bc)
                slabs[j] = None
                issue_norm_mult(j)

            def issue_norm_mult(j):
                if j not in pending:
                    return
                po, bc = pending.pop(j)
                h, qh = j // 2, j % 2
                t = h // 2
                qsl = slice(qh * JW, (qh + 1) * JW)
                if h % 2 == 0:
                    nc.vector.tensor_mul(otp[t][0:HD, qsl], po[0:HD, :], bc[:])
                else:
                    tmp = shift_pool.tile([HD, JW], BF16, tag="tmp")
                    nc.vector.tensor_mul(tmp[:], po[0:HD, :], bc[:])
                    nc.sync.dma_start(otp[t][HD:128, qsl], tmp[:])

            # -- prefix: QK slot 0, then V (PE-dense, exp engines idle) --
            qk_stack = ExitStack()
            qk_psum = qk_stack.enter_context(
                tc.tile_pool(name="qk_psum", bufs=2, space="PSUM")
            )
            for w_sb, dst in ((wq_sb, qt8), (wk_sb, kt8)):
                for qq in range(4):
                    for _ in qk_quarter(0, w_sb, dst, qq):
                        pass
            v_stack = ExitStack()
            v_psum = v_stack.enter_context(
                tc.tile_pool(name="v_psum", bufs=2, space="PSUM")
            )
            for si in range(NKC):
                for _ in v_chunk(si, v_psum):
                    pass
            v_stack.close()

            # -- pipelined jobs --
            # fillers: QK slots 1..3 emitted between score chunks of jobs 0..3
            fillers = []
            fillers_budget = [0.0]
            for t in range(1, GH // 2):
                for w_sb, dst in ((wq_sb, qt8), (wk_sb, kt8)):
                    for qq in range(4):
                        fillers.append(qk_quarter(t, w_sb, dst, qq))

            o_stack = ExitStack()
            o_psum = None
            for j in range(2 * GH):
                if j < PV_LAG:
                    pattern = _PAT_EARLY
                else:
                    pattern = _PAT_STEADY if j % 2 == 0 else _PAT_STEADY2
                issue_job_scores(j, fillers, pattern)
                if j == PV_LAG - 1:
                    # drain remaining QK fillers; free their psum for PV
                    while fillers:
                        try:
                            next(fillers[0])
                        except StopIteration:
                            fillers.pop(0)
                    qk_stack.close()
                    o_psum = o_stack.enter_context(
                        tc.tile_pool(name="o_psum", bufs=1, space="PSUM")
                    )
                    s_x_stack = ExitStack()
                    s_x = s_x_stack.enter_context(
                        tc.tile_pool(name="s_x", bufs=1, space="PSUM")
                    )
                    s_pools.extend([s_psum, s_x])
                if j >= PV_LAG:
                    issue_pv_den(j - PV_LAG, o_psum)
            # trailing PVs: reuse s_x's banks as a second PV buffer so the
            # per-job den/mult chains overlap
            s_x_stack.close()
            o2_stack = ExitStack()
            o2 = o2_stack.enter_context(
                tc.tile_pool(name="o2_psum", bufs=1, space="PSUM")
            )
            for i, j in enumerate(range(2 * GH - PV_LAG, 2 * GH)):
                issue_pv_den(j, o2 if i % 2 == 0 else o_psum)
                issue_norm_mult(j - 1)
            issue_norm_mult(2 * GH - 1)
            o2_stack.close()
            o_stack.close()
        phase1.close()

        # ---- output projection ------------------------------------------
        with tc.tile_pool(name="out_psum", bufs=2, space="PSUM") as out_psum, \
             tc.tile_pool(name="y", bufs=2) as y_pool:
            for ec in range(NDC):
                ps = out_psum.tile([128, S], F32, tag="yp")
                for mt in range(GM // 128):
                    lhsT = wo_sb[:, mt * D + ec * 128 : mt * D + (ec + 1) * 128]
                    for qb in range(4):
                        nc.tensor.matmul(
                            ps[:, qb * 512 : (qb + 1) * 512],
                            lhsT,
                            otp[mt][:, qb * 512 : (qb + 1) * 512],
                            start=(mt == 0),
                            stop=(mt == GM // 128 - 1),
                        )
                y_sb = y_pool.tile([128, S], F32, tag="y")
                if ec % 2 == 0:
                    nc.vector.tensor_scalar_add(
                        y_sb[:], ps[:], bo_sb[:, ec : ec + 1]
                    )
                else:
                    nc.scalar.activation(
                        y_sb[:], ps[:], AF.Identity, bias=bo_sb[:, ec : ec + 1]
                    )
                nc.sync.dma_start(outT[ec * 128 : (ec + 1) * 128, :], y_sb[:])

    return nc


_NC = None
_last_in_maps = None


def _get_program():
    global _NC
    if _NC is None:
        _NC = _build_program()
    return _NC


def make_in_maps(x, Wq, bq, Wk, bk, Wv, bv, Wo, bo):
    x = np.asarray(x, np.float32)
    bf = ml_dtypes.bfloat16
    in_maps = []
    for c in range(8):
        b, g = c // 2, c % 2
        sl = slice(g * GM, (g + 1) * GM)
        wo_slice = np.asarray(Wo, np.float32)[:, sl].T  # [512, 1024]
        # fold bv and half of bo into the output bias
        bo_eff = np.asarray(bo, np.float32) / 2.0 + np.asarray(bv, np.float32)[sl] @ wo_slice
        in_maps.append(
            {
                "xT": np.ascontiguousarray(x[b].T).astype(bf),
                "wq": np.ascontiguousarray(np.asarray(Wq, np.float32)[sl, :].T).astype(bf),
                "wk": np.ascontiguousarray(np.asarray(Wk, np.float32)[sl, :].T).astype(bf),
                "wv": np.ascontiguousarray(np.asarray(Wv, np.float32)[sl, :].T).astype(bf),
                "wo": np.ascontiguousarray(
                    wo_slice.reshape(GM // 128, 128, D).transpose(1, 0, 2).reshape(128, (GM // 128) * D)
                ).astype(bf),
                "bq": np.ascontiguousarray(np.asarray(bq, np.float32)[sl]),
                "bo": np.ascontiguousarray(bo_eff.astype(np.float32)),
            }
        )
    return in_maps


def expected_partial(c, x, Wq, bq, Wk, bk, Wv, bv, Wo, bo):
    """Numpy recomputation of core c's partial outT [D, S] (f32)."""
    b, g = c // 2, c % 2
    sl = slice(g * GM, (g + 1) * GM)
    xb = np.asarray(x, np.float32)[b]  # [S, D]
    Q = xb @ np.asarray(Wq, np.float32)[sl, :].T + np.asarray(bq, np.float32)[sl]
    K_ = xb @ np.asarray(Wk, np.float32)[sl, :].T + np.asarray(bk, np.float32)[sl]
    V = xb @ np.asarray(Wv, np.float32)[sl, :].T + np.asarray(bv, np.float32)[sl]
    out = np.empty((S, GM), np.float32)
    for h in range(GH):
        hs = slice(h * HD, (h + 1) * HD)
        sc = Q[:, hs] @ K_[:, hs].T / np.sqrt(HD)
        e = np.exp(sc - sc.max(-1, keepdims=True))
        out[:, hs] = (e @ V[:, hs]) / e.sum(-1, keepdims=True)
    y = out @ np.asarray(Wo, np.float32)[:, sl].T + np.asarray(bo, np.float32) / 2.0
    return np.ascontiguousarray(y.T)  # [D, S]


def kernel(x, Wq, bq, Wk, bk, Wv, bv, Wo, bo):
    in_maps = make_in_maps(x, Wq, bq, Wk, bk, Wv, bv, Wo, bo)
    global _last_in_maps
    _last_in_maps = in_maps
    nc = _get_program()
    res = run_bass_kernel_spmd(nc, in_maps, core_ids=list(range(8)))
    out = np.empty((B, S, D), np.float32)
    for b in range(B):
        acc = res.results[2 * b]["outT"].astype(np.float32) + res.results[
            2 * b + 1
        ]["outT"].astype(np.float32)
        out[b] = acc.T
    return out


# revision 62
# speedup vs baseline: 1.0079x; 1.0079x over previous
"""Multi-head self-attention (B=4, S=2048, D=1024, H=16, Hd=64) on 8 TRN2 cores.

Sharding: core c -> (batch b = c//2, head-group g = c%2 of 8 heads).
Each core computes its batch's 8 heads end-to-end plus the partial output
projection for its head group; the host sums the two head-group partials
per batch. No collectives.

Device layout is fully transposed: activations are [feature(partitions),
seq(free)]. QKV and output projections run in bf16. The attention part
(scores QK^T and PV) runs in fp8e4m3 with DoubleRow perf mode at 0.5
cycles/row. For scores, the two DoubleRow k-subtiles read the SAME 64-dim
Q/K data via a stride-0 middle dim (computing 2*QK^T); the doubling is
folded into the exp scale (1/16 instead of 1/8). V is stored as
[128 keys x kc-pair x 8 head x 65] with an all-ones 65th column producing
the softmax denominator inside the PV psum.

exp() splits across the Activation engine (exact exp, fp8 output) and the
DVE (Schraudolph: fp8e4m3 bits = trunc(s*scale*8/ln2 + 56.05) via one
tensor_scalar mult+add writing int8). Attention is pipelined as 16 jobs
(head, q-half) with PV lagging 4 jobs; Q/K projection m-tiles 1..3 are
emitted piecewise between score chunks so the PE fills exp-bound gaps.
K's bias is dropped (softmax-invariant); V's bias is folded into the
output-projection bias on the host.
"""

from contextlib import ExitStack

import numpy as np
import ml_dtypes

import concourse.bass as bass
import concourse.tile as tile
from concourse import mybir
from concourse.bass_utils import run_bass_kernel_spmd
from concourse.vector_clock import ScopedClock
from bass_rust import InstNoOp, SyncInfo

BF16 = mybir.dt.bfloat16
F32 = mybir.dt.float32
FP8 = mybir.dt.float8e4
I8 = mybir.dt.int8
AF = mybir.ActivationFunctionType
ALU = mybir.AluOpType
DR = mybir.MatmulPerfMode.DoubleRow

B, S, D = 4, 2048, 1024
H, HD = 16, 64
GH = 8          # heads per core (head-group size)
GM = GH * HD    # 512 head dims per core
NDC = 8         # d chunks of 128 (contraction for projections)
NKC = 16        # k chunks of 128
VW = HD + 1     # 65: per-head V columns + ones column
VWP = HD + 2    # 66: v8 stores heads 2-byte aligned (dual-fp8 LW requires
                # even weight byte offsets; 65-wide heads would give odd ones)
JW = S // 2     # 1024: q-width of one attention job
PV_LAG = 4      # jobs between scores(j) and PV(j)

# scores psum holds 2*QK^T (stride-0 DoubleRow pair), so exp scale is 1/16.
EXP_SCALE = 0.0625
SCH_MUL = EXP_SCALE * 8.0 / float(np.log(2.0))
SCH_ADD = 56.05


def _mk_pattern(counts):
    accs = {k: 0.0 for k in counts}
    total = sum(counts.values())
    out = []
    for _ in range(total):
        for k in counts:
            accs[k] += counts[k] / total
        k = max(accs, key=lambda q: accs[q])
        accs[k] -= 1.0
        out.append(k)
    return out


# exp engine per (kc) chunk within a job: 'A' = Act exact, 'D' = DVE
# Schraudolph. Early jobs carry QK copies on Act + V copies on DVE; later
# jobs carry norm work on DVE.
_PAT_EARLY = _mk_pattern({"A": 8, "D": 8})
_PAT_STEADY = _mk_pattern({"A": 10, "D": 6})
_PAT_STEADY2 = _mk_pattern({"A": 9, "D": 7})

_META_TYPES = ("TileBranchInst", "BassTileLoopBlock", "BassTilePoolBoundary")


class _TileCtx(tile.TileContext):
    """Splits multi-sem-wait instructions: the pinned walrus rejects any TPB
    instruction carrying more than one sem-wait, while Tile emits joins and a
    global end-of-context drain with several."""

    def _split_waits(self, ordered):
        nc = self.nc
        for bb_name, insts in ordered.items():
            out = []
            for inst in insts:
                si = inst.sync_info
                if (
                    si is not None
                    and si.on_wait
                    and len(si.on_wait) > 1
                    and type(inst).__name__ not in _META_TYPES
                    and inst.engine != mybir.EngineType.Unassigned
                ):
                    waits = list(si.on_wait)
                    for w in waits[:-1]:
                        nop = InstNoOp(
                            name=nc.get_next_instruction_name(), ins=[], outs=[]
                        )
                        nop.engine = inst.engine
                        nop.sync_info = SyncInfo(on_wait=[w], on_update=[])
                        out.append(nop)
                    inst.sync_info = SyncInfo(
                        on_wait=[waits[-1]], on_update=list(si.on_update)
                    )
                out.append(inst)
            ordered[bb_name] = out

    def _lower_ordered_insts(self, ordered):
        self._split_waits(ordered)
        super()._lower_ordered_insts(ordered)

    def _drain_and_barrier(self, tick_clock, wait_clock):
        drain_inst = self.nc.sync.drain()
        wait_clock.add_sem_waits(
            drain_inst.ins, ScopedClock({None: tick_clock.global_clock})
        )
        si = drain_inst.ins.sync_info
        waits = list(si.on_wait) if si is not None else []
        if len(waits) > 1:
            drain_inst.ins.sync_info = SyncInfo(
                on_wait=waits[:1], on_update=list(si.on_update)
            )
            for w in waits[1:]:
                extra = self.nc.sync.drain()
                extra.ins.sync_info = SyncInfo(on_wait=[w], on_update=[])

        self.nc.all_engine_barrier()
        assert self.sems is not None
        popped = self.nc._tile_sem_poison_stack.pop()
        assert popped is self._sem_poison
        self.nc.clear_and_free_semaphores(list(self.sems.allocated().values()))
        self.nc.all_engine_barrier()


def _build_program():
    nc = bass.Bass(trn_type="TRN2", debug=False, num_devices=8)

    xT = nc.dram_tensor("xT", [D, S], BF16, kind="ExternalInput").ap()
    wq = nc.dram_tensor("wq", [D, GM], BF16, kind="ExternalInput").ap()
    wk = nc.dram_tensor("wk", [D, GM], BF16, kind="ExternalInput").ap()
    wv = nc.dram_tensor("wv", [D, GM], BF16, kind="ExternalInput").ap()
    # pair-major-reordered Wo.T slice: [128, 4 pairs x 1024]
    wo = nc.dram_tensor("wo", [128, (GM // 128) * D], BF16, kind="ExternalInput").ap()
    bq = nc.dram_tensor("bq", [GM], F32, kind="ExternalInput").ap()
    bo = nc.dram_tensor("bo", [D], F32, kind="ExternalInput").ap()
    outT = nc.dram_tensor("outT", [D, S], F32, kind="ExternalOutput").ap()

    with _TileCtx(nc) as tc, ExitStack() as ctx:
        const_pool = ctx.enter_context(tc.tile_pool(name="const", bufs=1))
        act_pool = ctx.enter_context(tc.tile_pool(name="acts", bufs=1))

        # ---- constants / weights / inputs -------------------------------
        bq_sb = const_pool.tile([128, GM // 128], F32, tag="bq")
        nc.sync.dma_start(bq_sb[:], bq.rearrange("(c p) -> p c", p=128))
        bo_sb = const_pool.tile([128, NDC], F32, tag="bo")
        nc.sync.dma_start(bo_sb[:], bo.rearrange("(c p) -> p c", p=128))
        wo_sb = const_pool.tile([128, (GM // 128) * D], BF16, tag="wo")
        nc.sync.dma_start(wo_sb[:], wo[:, :])

        # persistent activations. qt8/kt8: [128, slot t, S]; head h lives at
        # partitions [64*(h%2), +64) of slot h//2.
        qt8 = act_pool.tile([128, GH // 2, S], FP8, tag="qt8")
        kt8 = act_pool.tile([128, GH // 2, S], FP8, tag="kt8")
        v8 = act_pool.tile([128, NKC // 2, 2, GH, VWP], FP8, tag="v8")
        nc.gpsimd.memset(v8[:, :, :, :, HD:VWP], 1.0)
        otp = [
            act_pool.tile([128, S], BF16, name=f"otp{t}", tag=f"otp{t}")
            for t in range(GH // 2)
        ]

        phase1 = ExitStack()
        w_pool = phase1.enter_context(tc.tile_pool(name="wts", bufs=1))
        xt = w_pool.tile([128, NDC * S], BF16, tag="xt")
        for t in range(NDC):
            eng = nc.sync if t % 2 == 0 else nc.scalar
            eng.dma_start(
                xt[:, t * S : (t + 1) * S], xT[t * 128 : (t + 1) * 128, :]
            )
        wv_sb = w_pool.tile([128, NDC * GM], BF16, tag="wv")
        nc.scalar.dma_start(
            wv_sb[:].rearrange("p (c m) -> p c m", m=GM),
            wv.rearrange("(c p) m -> p c m", p=128),
        )
        wq_sb = w_pool.tile([128, NDC * GM], BF16, tag="wq")
        nc.sync.dma_start(
            wq_sb[:].rearrange("p (c m) -> p c m", m=GM),
            wq.rearrange("(c p) m -> p c m", p=128),
        )
        wk_sb = w_pool.tile([128, NDC * GM], BF16, tag="wk")
        nc.sync.dma_start(
            wk_sb[:].rearrange("p (c m) -> p c m", m=GM),
            wk.rearrange("(c p) m -> p c m", p=128),
        )

        # ---- attention + projections, software-pipelined ----------------
        with tc.tile_pool(name="s_psum", bufs=2, space="PSUM") as s_psum, \
             tc.tile_pool(name="slab", bufs=PV_LAG + 1) as slab_pool, \
             tc.tile_pool(name="norm", bufs=1) as norm_pool, \
             tc.tile_pool(name="bcast", bufs=2) as bc_pool, \
             tc.tile_pool(name="shift", bufs=1) as shift_pool, \
             tc.tile_pool(name="dscr", bufs=2, space="DRAM") as dram_pool:

            def qk_quarter(t, w_sb, dst, qq):
                """Generator: Q or K projection m-tile t, q-quarter qq (512).
                Yields after small matmul groups so callers can interleave."""
                ps = qk_psum.tile([128, 512], F32, tag="qkp")
                q0 = qq * 512
                for dc in range(NDC):
                    nc.tensor.matmul(
                        ps[:],
                        w_sb[:, dc * GM + t * 128 : dc * GM + (t + 1) * 128],
                        xt[:, dc * S + q0 : dc * S + q0 + 512],
                        start=(dc == 0),
                        stop=(dc == NDC - 1),
                    )
                    if dc % 2 == 1:
                        yield
                if dst is qt8:
                    nc.scalar.activation(
                        dst[:, t, q0 : q0 + 512], ps[:], AF.Identity,
                        bias=bq_sb[:, t : t + 1],
                    )
                else:
                    nc.scalar.activation(dst[:, t, q0 : q0 + 512], ps[:], AF.Copy)
                yield

            def v_chunk(si, v_psum):
                """Generator: V projection keys-chunk si -> v8 (fp8)."""
                ps = v_psum.tile([128, GM], F32, tag="vp")
                for dc in range(NDC):
                    nc.tensor.matmul(
                        ps[:],
                        xt[:, dc * S + si * 128 : dc * S + (si + 1) * 128],
                        wv_sb[:, dc * GM : (dc + 1) * GM],
                        start=(dc == 0),
                        stop=(dc == NDC - 1),
                    )
                    if dc % 4 == 3:
                        yield
                nc.vector.tensor_copy(
                    v8[:, si // 2, si % 2, :, 0:HD],
                    ps[:].rearrange("p (h d) -> p h d", h=GH),
                )
                yield

            # jobs: j = 2*h + qh
            slabs = [None] * (2 * GH)

            s_pools = [s_psum]
            chunk_ctr = [0]

            def issue_job_scores(j, fillers, pattern, inject=None):
                """Scores+exp for job j, pulling filler matmul groups from
                `fillers` (a list of active generators) between chunks.
                `inject` maps chunk index -> thunk issued at that point."""
                h, qh = j // 2, j % 2
                slot, p0 = h // 2, 64 * (h % 2)
                sl = slab_pool.tile([128, NKC // 2, 2, JW], FP8, tag="slab")
                slabs[j] = sl
                psl = slice(p0, p0 + 64)
                for kc in range(NKC):
                    if inject and kc in inject:
                        inject[kc]()
                    u, jj = kc // 2, kc % 2
                    # kt8/qt8 [64, 1, N] viewed as a stride-0 [64, 2, N]
                    lhsT = kt8[psl, slot, kc * 128 : (kc + 1) * 128] \
                        .unsqueeze(1).broadcast_to([64, 2, 128])
                    sp = s_pools[chunk_ctr[0] % len(s_pools)].tile(
                        [128, JW], F32, tag="sp"
                    )
                    chunk_ctr[0] += 1
                    for qq in range(JW // 512):
                        q0 = qh * JW + qq * 512
                        rhs = qt8[psl, slot, q0 : q0 + 512] \
                            .unsqueeze(1).broadcast_to([64, 2, 512])
                        nc.tensor.matmul(
                            sp[:, qq * 512 : (qq + 1) * 512],
                            lhsT,
                            rhs,
                            start=True,
                            stop=True,
                            perf_mode=DR,
                        )
                    dst = sl[:, u, jj, :]
                    if pattern[kc] == "A":
                        nc.scalar.activation(dst, sp[:], AF.Exp, scale=EXP_SCALE)
                    else:
                        nc.vector.tensor_scalar(
                            dst.bitcast(I8), sp[:],
                            SCH_MUL, SCH_ADD, ALU.mult, ALU.add,
                        )
                    # pull ~1.4 filler steps per chunk so the 12 QK halves
                    # (slots 1-3) finish by the end of job 4
                    fillers_budget[0] += 1.4
                    while fillers and fillers_budget[0] >= 1.0:
                        try:
                            next(fillers[0])
                            fillers_budget[0] -= 1.0
                        except StopIteration:
                            fillers.pop(0)

            pending = {}  # j -> (po tile, bc tile)

            def issue_pv_den(j, o_psum):
                """PV matmuls + reciprocal + the den-broadcast DMA roundtrip.
                The otp multiply is deferred (issue_norm_mult) so the DVE
                never stalls in-order on the broadcast DMA."""
                h = j // 2
                sl = slabs[j]
                po = o_psum.tile([128, JW], F32, tag="op")
                for u in range(NKC // 2):
                    lhsT = v8[:, u, :, h, 0:VW]
                    for qs in range(JW // 512):
                        nc.tensor.matmul(
                            po[0:VW, qs * 512 : (qs + 1) * 512],
                            lhsT,
                            sl[:, u, :, qs * 512 : (qs + 1) * 512],
                            start=(u == 0),
                            stop=(u == NKC // 2 - 1),
                            perf_mode=DR,
                        )
                nrm = norm_pool.tile([VW, JW], F32, tag="nrm")
                nc.vector.reciprocal(nrm[HD:VW, :], po[HD:VW, :])
                scr = dram_pool.tile([JW], F32, tag="scr")
                nc.sync.dma_start(scr.unsqueeze(0), nrm[HD:VW, :])
                bc = bc_pool.tile([HD, JW], F32, tag="bc")
                nc.sync.dma_start(bc[:], scr.unsqueeze(0).broadcast_to([HD, JW]))
                pending[j] = (po, bc)
                slabs[j] = None
                issue_norm_mult(j)

            def issue_norm_mult(j):
                if j not in pending:
                    return
                po, bc = pending.pop(j)
                h, qh = j // 2, j % 2
                t = h // 2
                qsl = slice(qh * JW, (qh + 1) * JW)
                if h % 2 == 0:
                    nc.vector.tensor_mul(otp[t][0:HD, qsl], po[0:HD, :], bc[:])
                else:
                    tmp = shift_pool.tile([HD, JW], BF16, tag="tmp")
                    nc.vector.tensor_mul(tmp[:], po[0:HD, :], bc[:])
                    nc.sync.dma_start(otp[t][HD:128, qsl], tmp[:])

            # -- prefix: QK slot 0, then V (PE-dense, exp engines idle) --
            qk_stack = ExitStack()
            qk_psum = qk_stack.enter_context(
                tc.tile_pool(name="qk_psum", bufs=2, space="PSUM")
            )
            for w_sb, dst in ((wq_sb, qt8), (wk_sb, kt8)):
                for qq in range(4):
                    for _ in qk_quarter(0, w_sb, dst, qq):
                        pass
            v_stack = ExitStack()
            v_psum = v_stack.enter_context(
                tc.tile_pool(name="v_psum", bufs=2, space="PSUM")
            )
            for si in range(NKC):
                for _ in v_chunk(si, v_psum):
                    pass
            v_stack.close()

            # -- pipelined jobs --
            # fillers: QK slots 1..3 emitted between score chunks of jobs 0..3
            fillers = []
            fillers_budget = [0.0]
            for t in range(1, GH // 2):
                for w_sb, dst in ((wq_sb, qt8), (wk_sb, kt8)):
                    for qq in range(4):
                        fillers.append(qk_quarter(t, w_sb, dst, qq))

            o_stack = ExitStack()
            o_psum = None
            for j in range(2 * GH):
                if j < PV_LAG:
                    pattern = _PAT_EARLY
                else:
                    pattern = _PAT_STEADY if j % 2 == 0 else _PAT_STEADY2
                issue_job_scores(j, fillers, pattern)
                if j == PV_LAG - 1:
                    # drain remaining QK fillers; free their psum for PV
                    while fillers:
                        try:
                            next(fillers[0])
                        except StopIteration:
                            fillers.pop(0)
                    qk_stack.close()
                    o_psum = o_stack.enter_context(
                        tc.tile_pool(name="o_psum", bufs=1, space="PSUM")
                    )
                    s_x_stack = ExitStack()
                    s_x = s_x_stack.enter_context(
                        tc.tile_pool(name="s_x", bufs=1, space="PSUM")
                    )
                    s_pools.extend([s_psum, s_x])
                if j >= PV_LAG:
                    issue_pv_den(j - PV_LAG, o_psum)
            # trailing PVs: reuse s_x's banks as a second PV buffer so the
            # per-job den/mult chains overlap. The output projection runs
            # interleaved, borrowing idle scores-psum tiles: per (ec, qh)
            # half, the mt0-2 accumulation pre-runs during the trailing
            # chains and only the mt3 matmuls wait on the final norm.
            s_x_stack.close()
            o2_stack = ExitStack()
            o2 = o2_stack.enter_context(
                tc.tile_pool(name="o2_psum", bufs=1, space="PSUM")
            )
            y_pool = o2_stack.enter_context(tc.tile_pool(name="y", bufs=2))

            out_tiles = {}

            def out_prework(i):
                ec, qh = i // 2, i % 2
                ps = s_psum.tile([128, JW], F32, tag="sp")
                out_tiles[i] = ps
                for mt in range(GM // 128 - 1):
                    lhsT = wo_sb[:, mt * D + ec * 128 : mt * D + (ec + 1) * 128]
                    for qb in range(2):
                        q0 = qh * JW + qb * 512
                        nc.tensor.matmul(
                            ps[:, qb * 512 : (qb + 1) * 512],
                            lhsT,
                            otp[mt][:, q0 : q0 + 512],
                            start=(mt == 0),
                            stop=False,
                        )

            def out_finish(i):
                ec, qh = i // 2, i % 2
                ps = out_tiles.pop(i)
                mt = GM // 128 - 1
                lhsT = wo_sb[:, mt * D + ec * 128 : mt * D + (ec + 1) * 128]
                for qb in range(2):
                    q0 = qh * JW + qb * 512
                    nc.tensor.matmul(
                        ps[:, qb * 512 : (qb + 1) * 512],
                        lhsT,
                        otp[mt][:, q0 : q0 + 512],
                        start=False,
                        stop=True,
                    )
                y_sb = y_pool.tile([128, JW], F32, tag="y")
                nc.scalar.activation(
                    y_sb[:], ps[:], AF.Identity, bias=bo_sb[:, ec : ec + 1]
                )
                eng = nc.sync if i % 2 == 0 else nc.scalar
                eng.dma_start(
                    outT[ec * 128 : (ec + 1) * 128, qh * JW : (qh + 1) * JW],
                    y_sb[:],
                )

            for i, j in enumerate(range(2 * GH - PV_LAG, 2 * GH)):
                issue_pv_den(j, o2 if i % 2 == 0 else o_psum)
                issue_norm_mult(j - 1)
            out_prework(0)
            out_prework(1)
            issue_norm_mult(2 * GH - 1)
            for i in range(2 * NDC):
                out_finish(i)
                if i + 2 < 2 * NDC:
                    out_prework(i + 2)
            o2_stack.close()
            o_stack.close()
        phase1.close()

    return nc


_NC = None
_last_in_maps = None


def _get_program():
    global _NC
    if _NC is None:
        _NC = _build_program()
    return _NC


def make_in_maps(x, Wq, bq, Wk, bk, Wv, bv, Wo, bo):
    x = np.asarray(x, np.float32)
    bf = ml_dtypes.bfloat16
    in_maps = []
    for c in range(8):
        b, g = c // 2, c % 2
        sl = slice(g * GM, (g + 1) * GM)
        wo_slice = np.asarray(Wo, np.float32)[:, sl].T  # [512, 1024]
        # fold bv and half of bo into the output bias
        bo_eff = np.asarray(bo, np.float32) / 2.0 + np.asarray(bv, np.float32)[sl] @ wo_slice
        in_maps.append(
            {
                "xT": np.ascontiguousarray(x[b].T).astype(bf),
                "wq": np.ascontiguousarray(np.asarray(Wq, np.float32)[sl, :].T).astype(bf),
                "wk": np.ascontiguousarray(np.asarray(Wk, np.float32)[sl, :].T).astype(bf),
                "wv": np.ascontiguousarray(np.asarray(Wv, np.float32)[sl, :].T).astype(bf),
                "wo": np.ascontiguousarray(
                    wo_slice.reshape(GM // 128, 128, D).transpose(1, 0, 2).reshape(128, (GM // 128) * D)
                ).astype(bf),
                "bq": np.ascontiguousarray(np.asarray(bq, np.float32)[sl]),
                "bo": np.ascontiguousarray(bo_eff.astype(np.float32)),
            }
        )
    return in_maps


def expected_partial(c, x, Wq, bq, Wk, bk, Wv, bv, Wo, bo):
    """Numpy recomputation of core c's partial outT [D, S] (f32)."""
    b, g = c // 2, c % 2
    sl = slice(g * GM, (g + 1) * GM)
    xb = np.asarray(x, np.float32)[b]  # [S, D]
    Q = xb @ np.asarray(Wq, np.float32)[sl, :].T + np.asarray(bq, np.float32)[sl]
    K_ = xb @ np.asarray(Wk, np.float32)[sl, :].T + np.asarray(bk, np.float32)[sl]
    V = xb @ np.asarray(Wv, np.float32)[sl, :].T + np.asarray(bv, np.float32)[sl]
    out = np.empty((S, GM), np.float32)
    for h in range(GH):
        hs = slice(h * HD, (h + 1) * HD)
        sc = Q[:, hs] @ K_[:, hs].T / np.sqrt(HD)
        e = np.exp(sc - sc.max(-1, keepdims=True))
        out[:, hs] = (e @ V[:, hs]) / e.sum(-1, keepdims=True)
    y = out @ np.asarray(Wo, np.float32)[:, sl].T + np.asarray(bo, np.float32) / 2.0
    return np.ascontiguousarray(y.T)  # [D, S]


def kernel(x, Wq, bq, Wk, bk, Wv, bv, Wo, bo):
    in_maps = make_in_maps(x, Wq, bq, Wk, bk, Wv, bv, Wo, bo)
    global _last_in_maps
    _last_in_maps = in_maps
    nc = _get_program()
    res = run_bass_kernel_spmd(nc, in_maps, core_ids=list(range(8)))
    out = np.empty((B, S, D), np.float32)
    for b in range(B):
        acc = res.results[2 * b]["outT"].astype(np.float32) + res.results[
            2 * b + 1
        ]["outT"].astype(np.float32)
        out[b] = acc.T
    return out


# revision 69
# speedup vs baseline: 1.0172x; 1.0092x over previous
"""Multi-head self-attention (B=4, S=2048, D=1024, H=16, Hd=64) on 8 TRN2 cores.

Sharding: core c -> (batch b = c//2, head-group g = c%2 of 8 heads).
Each core computes its batch's 8 heads end-to-end plus the partial output
projection for its head group; the host sums the two head-group partials
per batch. No collectives.

Device layout is fully transposed: activations are [feature(partitions),
seq(free)]. QKV and output projections run in bf16. The attention part
(scores QK^T and PV) runs in fp8e4m3 with DoubleRow perf mode at 0.5
cycles/row. For scores, the two DoubleRow k-subtiles read the SAME 64-dim
Q/K data via a stride-0 middle dim (computing 2*QK^T); the doubling is
folded into the exp scale (1/16 instead of 1/8). V is stored as
[128 keys x kc-pair x 8 head x 65] with an all-ones 65th column producing
the softmax denominator inside the PV psum.

exp() splits across the Activation engine (exact exp, fp8 output) and the
DVE (Schraudolph: fp8e4m3 bits = trunc(s*scale*8/ln2 + 56.05) via one
tensor_scalar mult+add writing int8). Attention is pipelined as 16 jobs
(head, q-half) with PV lagging 4 jobs; Q/K projection m-tiles 1..3 are
emitted piecewise between score chunks so the PE fills exp-bound gaps.
K's bias is dropped (softmax-invariant); V's bias is folded into the
output-projection bias on the host.
"""

from contextlib import ExitStack

import numpy as np
import ml_dtypes

import concourse.bass as bass
import concourse.tile as tile
from concourse import mybir
from concourse.bass_utils import run_bass_kernel_spmd
from concourse.vector_clock import ScopedClock
from bass_rust import InstNoOp, SyncInfo

BF16 = mybir.dt.bfloat16
F32 = mybir.dt.float32
FP8 = mybir.dt.float8e4
I8 = mybir.dt.int8
AF = mybir.ActivationFunctionType
ALU = mybir.AluOpType
DR = mybir.MatmulPerfMode.DoubleRow

B, S, D = 4, 2048, 1024
H, HD = 16, 64
GH = 8          # heads per core (head-group size)
GM = GH * HD    # 512 head dims per core
NDC = 8         # d chunks of 128 (contraction for projections)
NKC = 16        # k chunks of 128
VW = HD + 1     # 65: per-head V columns + ones column
VWP = HD + 2    # 66: v8 stores heads 2-byte aligned (dual-fp8 LW requires
                # even weight byte offsets; 65-wide heads would give odd ones)
JW = S // 2     # 1024: q-width of one attention job
PV_LAG = 4      # jobs between scores(j) and PV(j)

# scores psum holds 2*QK^T (stride-0 DoubleRow pair), so exp scale is 1/16.
EXP_SCALE = 0.0625
SCH_MUL = EXP_SCALE * 8.0 / float(np.log(2.0))
SCH_ADD = 56.05


def _mk_pattern(counts):
    accs = {k: 0.0 for k in counts}
    total = sum(counts.values())
    out = []
    for _ in range(total):
        for k in counts:
            accs[k] += counts[k] / total
        k = max(accs, key=lambda q: accs[q])
        accs[k] -= 1.0
        out.append(k)
    return out


# exp engine per (kc) chunk within a job: 'A' = Act exact, 'D' = DVE
# Schraudolph. Early jobs carry QK copies on Act + V copies on DVE; later
# jobs carry norm work on DVE.
_PAT_EARLY = _mk_pattern({"A": 8, "D": 8})
_PAT_STEADY = _mk_pattern({"A": 11, "D": 5})
_PAT_STEADY2 = _mk_pattern({"A": 9, "D": 7})

_META_TYPES = ("TileBranchInst", "BassTileLoopBlock", "BassTilePoolBoundary")


class _TileCtx(tile.TileContext):
    """Splits multi-sem-wait instructions: the pinned walrus rejects any TPB
    instruction carrying more than one sem-wait, while Tile emits joins and a
    global end-of-context drain with several."""

    def _split_waits(self, ordered):
        nc = self.nc
        for bb_name, insts in ordered.items():
            out = []
            for inst in insts:
                si = inst.sync_info
                if (
                    si is not None
                    and si.on_wait
                    and len(si.on_wait) > 1
                    and type(inst).__name__ not in _META_TYPES
                    and inst.engine != mybir.EngineType.Unassigned
                ):
                    waits = list(si.on_wait)
                    for w in waits[:-1]:
                        nop = InstNoOp(
                            name=nc.get_next_instruction_name(), ins=[], outs=[]
                        )
                        nop.engine = inst.engine
                        nop.sync_info = SyncInfo(on_wait=[w], on_update=[])
                        out.append(nop)
                    inst.sync_info = SyncInfo(
                        on_wait=[waits[-1]], on_update=list(si.on_update)
                    )
                out.append(inst)
            ordered[bb_name] = out

    def _lower_ordered_insts(self, ordered):
        self._split_waits(ordered)
        super()._lower_ordered_insts(ordered)

    def _drain_and_barrier(self, tick_clock, wait_clock):
        drain_inst = self.nc.sync.drain()
        wait_clock.add_sem_waits(
            drain_inst.ins, ScopedClock({None: tick_clock.global_clock})
        )
        si = drain_inst.ins.sync_info
        waits = list(si.on_wait) if si is not None else []
        if len(waits) > 1:
            drain_inst.ins.sync_info = SyncInfo(
                on_wait=waits[:1], on_update=list(si.on_update)
            )
            for w in waits[1:]:
                extra = self.nc.sync.drain()
                extra.ins.sync_info = SyncInfo(on_wait=[w], on_update=[])

        self.nc.all_engine_barrier()
        assert self.sems is not None
        popped = self.nc._tile_sem_poison_stack.pop()
        assert popped is self._sem_poison
        self.nc.clear_and_free_semaphores(list(self.sems.allocated().values()))
        self.nc.all_engine_barrier()


def _build_program():
    nc = bass.Bass(trn_type="TRN2", debug=False, num_devices=8)

    xT = nc.dram_tensor("xT", [D, S], BF16, kind="ExternalInput").ap()
    wq = nc.dram_tensor("wq", [D, GM], BF16, kind="ExternalInput").ap()
    wk = nc.dram_tensor("wk", [D, GM], BF16, kind="ExternalInput").ap()
    wv = nc.dram_tensor("wv", [D, GM], BF16, kind="ExternalInput").ap()
    # pair-major-reordered Wo.T slice: [128, 4 pairs x 1024]
    wo = nc.dram_tensor("wo", [128, (GM // 128) * D], BF16, kind="ExternalInput").ap()
    bq = nc.dram_tensor("bq", [GM], F32, kind="ExternalInput").ap()
    bo = nc.dram_tensor("bo", [D], F32, kind="ExternalInput").ap()
    outT = nc.dram_tensor("outT", [D, S], F32, kind="ExternalOutput").ap()

    with _TileCtx(nc) as tc, ExitStack() as ctx:
        const_pool = ctx.enter_context(tc.tile_pool(name="const", bufs=1))
        act_pool = ctx.enter_context(tc.tile_pool(name="acts", bufs=1))

        # ---- constants / weights / inputs -------------------------------
        bq_sb = const_pool.tile([128, GM // 128], F32, tag="bq")
        nc.sync.dma_start(bq_sb[:], bq.rearrange("(c p) -> p c", p=128))
        bo_sb = const_pool.tile([128, NDC], F32, tag="bo")
        nc.sync.dma_start(bo_sb[:], bo.rearrange("(c p) -> p c", p=128))
        wo_sb = const_pool.tile([128, (GM // 128) * D], BF16, tag="wo")
        nc.sync.dma_start(wo_sb[:], wo[:, :])

        # persistent activations. qt8/kt8: [128, slot t, S]; head h lives at
        # partitions [64*(h%2), +64) of slot h//2.
        qt8 = act_pool.tile([128, GH // 2, S], FP8, tag="qt8")
        kt8 = act_pool.tile([128, GH // 2, S], FP8, tag="kt8")
        v8 = act_pool.tile([128, NKC // 2, 2, GH, VWP], FP8, tag="v8")
        nc.gpsimd.memset(v8[:, :, :, :, HD:VWP], 1.0)
        otp = [
            act_pool.tile([128, S], BF16, name=f"otp{t}", tag=f"otp{t}")
            for t in range(GH // 2)
        ]

        phase1 = ExitStack()
        w_pool = phase1.enter_context(tc.tile_pool(name="wts", bufs=1))
        xt = w_pool.tile([128, NDC * S], BF16, tag="xt")
        for t in range(NDC):
            eng = nc.sync if t % 2 == 0 else nc.scalar
            eng.dma_start(
                xt[:, t * S : (t + 1) * S], xT[t * 128 : (t + 1) * 128, :]
            )
        wv_sb = w_pool.tile([128, NDC * GM], BF16, tag="wv")
        nc.scalar.dma_start(
            wv_sb[:].rearrange("p (c m) -> p c m", m=GM),
            wv.rearrange("(c p) m -> p c m", p=128),
        )
        wq_sb = w_pool.tile([128, NDC * GM], BF16, tag="wq")
        nc.sync.dma_start(
            wq_sb[:].rearrange("p (c m) -> p c m", m=GM),
            wq.rearrange("(c p) m -> p c m", p=128),
        )
        wk_sb = w_pool.tile([128, NDC * GM], BF16, tag="wk")
        nc.sync.dma_start(
            wk_sb[:].rearrange("p (c m) -> p c m", m=GM),
            wk.rearrange("(c p) m -> p c m", p=128),
        )

        # ---- attention + projections, software-pipelined ----------------
        with tc.tile_pool(name="s_psum", bufs=2, space="PSUM") as s_psum, \
             tc.tile_pool(name="slab", bufs=PV_LAG + 1) as slab_pool, \
             tc.tile_pool(name="norm", bufs=1) as norm_pool, \
             tc.tile_pool(name="bcast", bufs=2) as bc_pool, \
             tc.tile_pool(name="shift", bufs=1) as shift_pool, \
             tc.tile_pool(name="dscr", bufs=2, space="DRAM") as dram_pool:

            def qk_quarter(t, w_sb, dst, qq):
                """Generator: Q or K projection m-tile t, q-quarter qq (512).
                Yields after small matmul groups so callers can interleave."""
                ps = qk_psum.tile([128, 512], F32, tag="qkp")
                q0 = qq * 512
                for dc in range(NDC):
                    nc.tensor.matmul(
                        ps[:],
                        w_sb[:, dc * GM + t * 128 : dc * GM + (t + 1) * 128],
                        xt[:, dc * S + q0 : dc * S + q0 + 512],
                        start=(dc == 0),
                        stop=(dc == NDC - 1),
                    )
                    if dc % 2 == 1:
                        yield
                if dst is qt8:
                    nc.scalar.activation(
                        dst[:, t, q0 : q0 + 512], ps[:], AF.Identity,
                        bias=bq_sb[:, t : t + 1],
                    )
                else:
                    nc.scalar.activation(dst[:, t, q0 : q0 + 512], ps[:], AF.Copy)
                yield

            def v_chunk(si, v_psum):
                """Generator: V projection keys-chunk si -> v8 (fp8)."""
                ps = v_psum.tile([128, GM], F32, tag="vp")
                for dc in range(NDC):
                    nc.tensor.matmul(
                        ps[:],
                        xt[:, dc * S + si * 128 : dc * S + (si + 1) * 128],
                        wv_sb[:, dc * GM : (dc + 1) * GM],
                        start=(dc == 0),
                        stop=(dc == NDC - 1),
                    )
                    if dc % 4 == 3:
                        yield
                nc.vector.tensor_copy(
                    v8[:, si // 2, si % 2, :, 0:HD],
                    ps[:].rearrange("p (h d) -> p h d", h=GH),
                )
                yield

            # jobs: j = 2*h + qh
            slabs = [None] * (2 * GH)

            s_pools = [s_psum]
            chunk_ctr = [0]

            def issue_job_scores(j, fillers, pattern, inject=None):
                """Scores+exp for job j, pulling filler matmul groups from
                `fillers` (a list of active generators) between chunks.
                `inject` maps chunk index -> thunk issued at that point."""
                h, qh = j // 2, j % 2
                slot, p0 = h // 2, 64 * (h % 2)
                sl = slab_pool.tile([128, NKC // 2, 2, JW], FP8, tag="slab")
                slabs[j] = sl
                psl = slice(p0, p0 + 64)
                for kc in range(NKC):
                    if inject and kc in inject:
                        inject[kc]()
                    u, jj = kc // 2, kc % 2
                    # kt8/qt8 [64, 1, N] viewed as a stride-0 [64, 2, N]
                    lhsT = kt8[psl, slot, kc * 128 : (kc + 1) * 128] \
                        .unsqueeze(1).broadcast_to([64, 2, 128])
                    sp = s_pools[chunk_ctr[0] % len(s_pools)].tile(
                        [128, JW], F32, tag="sp"
                    )
                    chunk_ctr[0] += 1
                    for qq in range(JW // 512):
                        q0 = qh * JW + qq * 512
                        rhs = qt8[psl, slot, q0 : q0 + 512] \
                            .unsqueeze(1).broadcast_to([64, 2, 512])
                        nc.tensor.matmul(
                            sp[:, qq * 512 : (qq + 1) * 512],
                            lhsT,
                            rhs,
                            start=True,
                            stop=True,
                            perf_mode=DR,
                        )
                    dst = sl[:, u, jj, :]
                    if pattern[kc] == "A":
                        nc.scalar.activation(dst, sp[:], AF.Exp, scale=EXP_SCALE)
                    else:
                        nc.vector.tensor_scalar(
                            dst.bitcast(I8), sp[:],
                            SCH_MUL, SCH_ADD, ALU.mult, ALU.add,
                        )
                    # pull ~1.4 filler steps per chunk so the 12 QK halves
                    # (slots 1-3) finish by the end of job 4
                    fillers_budget[0] += 1.4
                    while fillers and fillers_budget[0] >= 1.0:
                        try:
                            next(fillers[0])
                            fillers_budget[0] -= 1.0
                        except StopIteration:
                            fillers.pop(0)

            pending = {}  # j -> (po tile, bc tile)

            def issue_pv_den(j, o_psum):
                """PV matmuls + reciprocal + the den-broadcast DMA roundtrip.
                The otp multiply is deferred (issue_norm_mult) so the DVE
                never stalls in-order on the broadcast DMA."""
                h = j // 2
                sl = slabs[j]
                po = o_psum.tile([128, JW], F32, tag="op")
                for u in range(NKC // 2):
                    lhsT = v8[:, u, :, h, 0:VW]
                    for qs in range(JW // 512):
                        nc.tensor.matmul(
                            po[0:VW, qs * 512 : (qs + 1) * 512],
                            lhsT,
                            sl[:, u, :, qs * 512 : (qs + 1) * 512],
                            start=(u == 0),
                            stop=(u == NKC // 2 - 1),
                            perf_mode=DR,
                        )
                nrm = norm_pool.tile([VW, JW], F32, tag="nrm")
                nc.vector.reciprocal(nrm[HD:VW, :], po[HD:VW, :])
                scr = dram_pool.tile([JW], F32, tag="scr")
                nc.sync.dma_start(scr.unsqueeze(0), nrm[HD:VW, :])
                bc = bc_pool.tile([HD, JW], F32, tag="bc")
                nc.sync.dma_start(bc[:], scr.unsqueeze(0).broadcast_to([HD, JW]))
                pending[j] = (po, bc)
                slabs[j] = None
                issue_norm_mult(j)

            def issue_norm_mult(j):
                if j not in pending:
                    return
                po, bc = pending.pop(j)
                h, qh = j // 2, j % 2
                t = h // 2
                qsl = slice(qh * JW, (qh + 1) * JW)
                if h % 2 == 0:
                    nc.vector.tensor_mul(otp[t][0:HD, qsl], po[0:HD, :], bc[:])
                else:
                    tmp = shift_pool.tile([HD, JW], BF16, tag="tmp")
                    nc.vector.tensor_mul(tmp[:], po[0:HD, :], bc[:])
                    nc.sync.dma_start(otp[t][HD:128, qsl], tmp[:])

            # -- prefix: QK slot 0, then V (PE-dense, exp engines idle) --
            qk_stack = ExitStack()
            qk_psum = qk_stack.enter_context(
                tc.tile_pool(name="qk_psum", bufs=2, space="PSUM")
            )
            for w_sb, dst in ((wq_sb, qt8), (wk_sb, kt8)):
                for qq in range(4):
                    for _ in qk_quarter(0, w_sb, dst, qq):
                        pass
            v_stack = ExitStack()
            v_psum = v_stack.enter_context(
                tc.tile_pool(name="v_psum", bufs=2, space="PSUM")
            )
            for si in range(NKC):
                for _ in v_chunk(si, v_psum):
                    pass
            v_stack.close()

            # -- pipelined jobs --
            # fillers: QK slots 1..3 emitted between score chunks of jobs 0..3
            fillers = []
            fillers_budget = [0.0]
            for t in range(1, GH // 2):
                for w_sb, dst in ((wq_sb, qt8), (wk_sb, kt8)):
                    for qq in range(4):
                        fillers.append(qk_quarter(t, w_sb, dst, qq))

            o_stack = ExitStack()
            o_psum = None
            for j in range(2 * GH):
                if j < PV_LAG:
                    pattern = _PAT_EARLY
                else:
                    pattern = _PAT_STEADY
                issue_job_scores(j, fillers, pattern)
                if j == PV_LAG - 1:
                    # drain remaining QK fillers; free their psum for PV
                    while fillers:
                        try:
                            next(fillers[0])
                        except StopIteration:
                            fillers.pop(0)
                    qk_stack.close()
                    o_psum = o_stack.enter_context(
                        tc.tile_pool(name="o_psum", bufs=1, space="PSUM")
                    )
                    s_x_stack = ExitStack()
                    s_x = s_x_stack.enter_context(
                        tc.tile_pool(name="s_x", bufs=1, space="PSUM")
                    )
                    s_pools.extend([s_psum, s_x])
                if j >= PV_LAG:
                    issue_pv_den(j - PV_LAG, o_psum)
            # trailing PVs: reuse s_x's banks as a second PV buffer so the
            # per-job den/mult chains overlap. The output projection runs
            # interleaved, borrowing idle scores-psum tiles: per (ec, qh)
            # half, the mt0-2 accumulation pre-runs during the trailing
            # chains and only the mt3 matmuls wait on the final norm.
            s_x_stack.close()
            o2_stack = ExitStack()
            o2 = o2_stack.enter_context(
                tc.tile_pool(name="o2_psum", bufs=1, space="PSUM")
            )
            y_pool = o2_stack.enter_context(tc.tile_pool(name="y", bufs=2))

            out_tiles = {}

            def out_prework(i):
                ec, qh = i // 2, i % 2
                ps = s_psum.tile([128, JW], F32, tag="sp")
                out_tiles[i] = ps
                for mt in range(GM // 128 - 1):
                    lhsT = wo_sb[:, mt * D + ec * 128 : mt * D + (ec + 1) * 128]
                    for qb in range(2):
                        q0 = qh * JW + qb * 512
                        nc.tensor.matmul(
                            ps[:, qb * 512 : (qb + 1) * 512],
                            lhsT,
                            otp[mt][:, q0 : q0 + 512],
                            start=(mt == 0),
                            stop=False,
                        )

            def out_finish(i):
                ec, qh = i // 2, i % 2
                ps = out_tiles.pop(i)
                mt = GM // 128 - 1
                lhsT = wo_sb[:, mt * D + ec * 128 : mt * D + (ec + 1) * 128]
                for qb in range(2):
                    q0 = qh * JW + qb * 512
                    nc.tensor.matmul(
                        ps[:, qb * 512 : (qb + 1) * 512],
                        lhsT,
                        otp[mt][:, q0 : q0 + 512],
                        start=False,
                        stop=True,
                    )
                y_sb = y_pool.tile([128, JW], F32, tag="y")
                nc.scalar.activation(
                    y_sb[:], ps[:], AF.Identity, bias=bo_sb[:, ec : ec + 1]
                )
                eng = nc.sync if i % 2 == 0 else nc.scalar
                eng.dma_start(
                    outT[ec * 128 : (ec + 1) * 128, qh * JW : (qh + 1) * JW],
                    y_sb[:],
                )

            for i, j in enumerate(range(2 * GH - PV_LAG, 2 * GH)):
                issue_pv_den(j, o2 if i % 2 == 0 else o_psum)
                issue_norm_mult(j - 1)
            out_prework(0)
            out_prework(1)
            issue_norm_mult(2 * GH - 1)
            for i in range(2 * NDC):
                out_finish(i)
                if i + 2 < 2 * NDC:
                    out_prework(i + 2)
            o2_stack.close()
            o_stack.close()
        phase1.close()

    return nc


_NC = None
_last_in_maps = None


def _get_program():
    global _NC
    if _NC is None:
        _NC = _build_program()
    return _NC


def make_in_maps(x, Wq, bq, Wk, bk, Wv, bv, Wo, bo):
    x = np.asarray(x, np.float32)
    bf = ml_dtypes.bfloat16
    in_maps = []
    for c in range(8):
        b, g = c // 2, c % 2
        sl = slice(g * GM, (g + 1) * GM)
        wo_slice = np.asarray(Wo, np.float32)[:, sl].T  # [512, 1024]
        # fold bv and half of bo into the output bias
        bo_eff = np.asarray(bo, np.float32) / 2.0 + np.asarray(bv, np.float32)[sl] @ wo_slice
        in_maps.append(
            {
                "xT": np.ascontiguousarray(x[b].T).astype(bf),
                "wq": np.ascontiguousarray(np.asarray(Wq, np.float32)[sl, :].T).astype(bf),
                "wk": np.ascontiguousarray(np.asarray(Wk, np.float32)[sl, :].T).astype(bf),
                "wv": np.ascontiguousarray(np.asarray(Wv, np.float32)[sl, :].T).astype(bf),
                "wo": np.ascontiguousarray(
                    wo_slice.reshape(GM // 128, 128, D).transpose(1, 0, 2).reshape(128, (GM // 128) * D)
                ).astype(bf),
                "bq": np.ascontiguousarray(np.asarray(bq, np.float32)[sl]),
                "bo": np.ascontiguousarray(bo_eff.astype(np.float32)),
            }
        )
    return in_maps


def expected_partial(c, x, Wq, bq, Wk, bk, Wv, bv, Wo, bo):
    """Numpy recomputation of core c's partial outT [D, S] (f32)."""
    b, g = c // 2, c % 2
    sl = slice(g * GM, (g + 1) * GM)
    xb = np.asarray(x, np.float32)[b]  # [S, D]
    Q = xb @ np.asarray(Wq, np.float32)[sl, :].T + np.asarray(bq, np.float32)[sl]
    K_ = xb @ np.asarray(Wk, np.float32)[sl, :].T + np.asarray(bk, np.float32)[sl]
    V = xb @ np.asarray(Wv, np.float32)[sl, :].T + np.asarray(bv, np.float32)[sl]
    out = np.empty((S, GM), np.float32)
    for h in range(GH):
        hs = slice(h * HD, (h + 1) * HD)
        sc = Q[:, hs] @ K_[:, hs].T / np.sqrt(HD)
        e = np.exp(sc - sc.max(-1, keepdims=True))
        out[:, hs] = (e @ V[:, hs]) / e.sum(-1, keepdims=True)
    y = out @ np.asarray(Wo, np.float32)[:, sl].T + np.asarray(bo, np.float32) / 2.0
    return np.ascontiguousarray(y.T)  # [D, S]


def kernel(x, Wq, bq, Wk, bk, Wv, bv, Wo, bo):
    in_maps = make_in_maps(x, Wq, bq, Wk, bk, Wv, bv, Wo, bo)
    global _last_in_maps
    _last_in_maps = in_maps
    nc = _get_program()
    res = run_bass_kernel_spmd(nc, in_maps, core_ids=list(range(8)))
    out = np.empty((B, S, D), np.float32)
    for b in range(B):
        acc = res.results[2 * b]["outT"].astype(np.float32) + res.results[
            2 * b + 1
        ]["outT"].astype(np.float32)
        out[b] = acc.T
    return out


# revision 79
# speedup vs baseline: 1.0196x; 1.0024x over previous
"""Multi-head self-attention (B=4, S=2048, D=1024, H=16, Hd=64) on 8 TRN2 cores.

Sharding: core c -> (batch b = c//2, head-group g = c%2 of 8 heads).
Each core computes its batch's 8 heads end-to-end plus the partial output
projection for its head group; the host sums the two head-group partials
per batch. No collectives.

Device layout is fully transposed: activations are [feature(partitions),
seq(free)]. QKV and output projections run in bf16. The attention part
(scores QK^T and PV) runs in fp8e4m3 with DoubleRow perf mode at 0.5
cycles/row. For scores, the two DoubleRow k-subtiles read the SAME 64-dim
Q/K data via a stride-0 middle dim (computing 2*QK^T); the doubling is
folded into the exp scale (1/16 instead of 1/8). V is stored as
[128 keys x kc-pair x 8 head x 65] with an all-ones 65th column producing
the softmax denominator inside the PV psum.

exp() splits across the Activation engine (exact exp, fp8 output) and the
DVE (Schraudolph: fp8e4m3 bits = trunc(s*scale*8/ln2 + 56.05) via one
tensor_scalar mult+add writing int8). Attention is pipelined as 16 jobs
(head, q-half) with PV lagging 4 jobs; Q/K projection m-tiles 1..3 are
emitted piecewise between score chunks so the PE fills exp-bound gaps.
K's bias is dropped (softmax-invariant); V's bias is folded into the
output-projection bias on the host.
"""

from contextlib import ExitStack

import numpy as np
import ml_dtypes

import concourse.bass as bass
import concourse.tile as tile
from concourse import mybir
from concourse.bass_utils import run_bass_kernel_spmd
from concourse.vector_clock import ScopedClock
from bass_rust import InstNoOp, SyncInfo

BF16 = mybir.dt.bfloat16
F32 = mybir.dt.float32
FP8 = mybir.dt.float8e4
I8 = mybir.dt.int8
AF = mybir.ActivationFunctionType
ALU = mybir.AluOpType
DR = mybir.MatmulPerfMode.DoubleRow

B, S, D = 4, 2048, 1024
H, HD = 16, 64
GH = 8          # heads per core (head-group size)
GM = GH * HD    # 512 head dims per core
NDC = 8         # d chunks of 128 (contraction for projections)
NKC = 16        # k chunks of 128
VW = HD + 1     # 65: per-head V columns + ones column
VWP = HD + 2    # 66: v8 stores heads 2-byte aligned (dual-fp8 LW requires
                # even weight byte offsets; 65-wide heads would give odd ones)
JW = S // 2     # 1024: q-width of one attention job
PV_LAG = 4      # jobs between scores(j) and PV(j)

# scores psum holds 2*QK^T (stride-0 DoubleRow pair), so exp scale is 1/16.
EXP_SCALE = 0.0625
SCH_MUL = EXP_SCALE * 8.0 / float(np.log(2.0))
SCH_ADD = 56.05


def _mk_pattern(counts):
    accs = {k: 0.0 for k in counts}
    total = sum(counts.values())
    out = []
    for _ in range(total):
        for k in counts:
            accs[k] += counts[k] / total
        k = max(accs, key=lambda q: accs[q])
        accs[k] -= 1.0
        out.append(k)
    return out


# exp engine per (kc) chunk within a job: 'A' = Act exact, 'D' = DVE
# Schraudolph. Early jobs carry QK copies on Act + V copies on DVE; later
# jobs carry norm work on DVE.
_PAT_EARLY = _mk_pattern({"A": 8, "D": 8})
_PAT_STEADY = _mk_pattern({"A": 11, "D": 5})
_PAT_STEADY2 = _mk_pattern({"A": 9, "D": 7})

_META_TYPES = ("TileBranchInst", "BassTileLoopBlock", "BassTilePoolBoundary")


class _TileCtx(tile.TileContext):
    """Splits multi-sem-wait instructions: the pinned walrus rejects any TPB
    instruction carrying more than one sem-wait, while Tile emits joins and a
    global end-of-context drain with several."""

    def _split_waits(self, ordered):
        nc = self.nc
        for bb_name, insts in ordered.items():
            out = []
            for inst in insts:
                si = inst.sync_info
                if (
                    si is not None
                    and si.on_wait
                    and len(si.on_wait) > 1
                    and type(inst).__name__ not in _META_TYPES
                    and inst.engine != mybir.EngineType.Unassigned
                ):
                    waits = list(si.on_wait)
                    for w in waits[:-1]:
                        nop = InstNoOp(
                            name=nc.get_next_instruction_name(), ins=[], outs=[]
                        )
                        nop.engine = inst.engine
                        nop.sync_info = SyncInfo(on_wait=[w], on_update=[])
                        out.append(nop)
                    inst.sync_info = SyncInfo(
                        on_wait=[waits[-1]], on_update=list(si.on_update)
                    )
                out.append(inst)
            ordered[bb_name] = out

    def _lower_ordered_insts(self, ordered):
        self._split_waits(ordered)
        super()._lower_ordered_insts(ordered)

    def _drain_and_barrier(self, tick_clock, wait_clock):
        drain_inst = self.nc.sync.drain()
        wait_clock.add_sem_waits(
            drain_inst.ins, ScopedClock({None: tick_clock.global_clock})
        )
        si = drain_inst.ins.sync_info
        waits = list(si.on_wait) if si is not None else []
        if len(waits) > 1:
            drain_inst.ins.sync_info = SyncInfo(
                on_wait=waits[:1], on_update=list(si.on_update)
            )
            for w in waits[1:]:
                extra = self.nc.sync.drain()
                extra.ins.sync_info = SyncInfo(on_wait=[w], on_update=[])

        self.nc.all_engine_barrier()
        assert self.sems is not None
        popped = self.nc._tile_sem_poison_stack.pop()
        assert popped is self._sem_poison
        self.nc.clear_and_free_semaphores(list(self.sems.allocated().values()))
        self.nc.all_engine_barrier()


def _build_program():
    nc = bass.Bass(trn_type="TRN2", debug=False, num_devices=8)

    xT = nc.dram_tensor("xT", [D, S], BF16, kind="ExternalInput").ap()
    wq = nc.dram_tensor("wq", [D, GM], BF16, kind="ExternalInput").ap()
    wk = nc.dram_tensor("wk", [D, GM], BF16, kind="ExternalInput").ap()
    wv = nc.dram_tensor("wv", [D, GM], BF16, kind="ExternalInput").ap()
    # pair-major-reordered Wo.T slice: [128, 4 pairs x 1024]
    wo = nc.dram_tensor("wo", [128, (GM // 128) * D], BF16, kind="ExternalInput").ap()
    bq = nc.dram_tensor("bq", [GM], F32, kind="ExternalInput").ap()
    bo = nc.dram_tensor("bo", [D], F32, kind="ExternalInput").ap()
    outT = nc.dram_tensor("outT", [D, S], F32, kind="ExternalOutput").ap()

    with _TileCtx(nc) as tc, ExitStack() as ctx:
        const_pool = ctx.enter_context(tc.tile_pool(name="const", bufs=1))
        act_pool = ctx.enter_context(tc.tile_pool(name="acts", bufs=1))

        # ---- constants / weights / inputs -------------------------------
        bq_sb = const_pool.tile([128, GM // 128], F32, tag="bq")
        nc.sync.dma_start(bq_sb[:], bq.rearrange("(c p) -> p c", p=128))
        bo_sb = const_pool.tile([128, NDC], F32, tag="bo")
        nc.sync.dma_start(bo_sb[:], bo.rearrange("(c p) -> p c", p=128))
        wo_sb = const_pool.tile([128, (GM // 128) * D], BF16, tag="wo")
        nc.sync.dma_start(wo_sb[:], wo[:, :])

        # persistent activations. qt8/kt8: [128, slot t, S]; head h lives at
        # partitions [64*(h%2), +64) of slot h//2.
        qt8 = act_pool.tile([128, GH // 2, S], FP8, tag="qt8")
        kt8 = act_pool.tile([128, GH // 2, S], FP8, tag="kt8")
        v8 = act_pool.tile([128, NKC // 2, 2, GH, VWP], FP8, tag="v8")
        nc.gpsimd.memset(v8[:, :, :, :, HD:VWP], 1.0)
        otp = [
            act_pool.tile([128, S], BF16, name=f"otp{t}", tag=f"otp{t}")
            for t in range(GH // 2)
        ]

        phase1 = ExitStack()
        w_pool = phase1.enter_context(tc.tile_pool(name="wts", bufs=1))
        xt = w_pool.tile([128, NDC * S], BF16, tag="xt")
        for t in range(NDC):
            eng = nc.sync if t % 2 == 0 else nc.scalar
            eng.dma_start(
                xt[:, t * S : (t + 1) * S], xT[t * 128 : (t + 1) * 128, :]
            )
        wv_sb = w_pool.tile([128, NDC * GM], BF16, tag="wv")
        nc.scalar.dma_start(
            wv_sb[:].rearrange("p (c m) -> p c m", m=GM),
            wv.rearrange("(c p) m -> p c m", p=128),
        )
        wq_sb = w_pool.tile([128, NDC * GM], BF16, tag="wq")
        nc.sync.dma_start(
            wq_sb[:].rearrange("p (c m) -> p c m", m=GM),
            wq.rearrange("(c p) m -> p c m", p=128),
        )
        wk_sb = w_pool.tile([128, NDC * GM], BF16, tag="wk")
        nc.sync.dma_start(
            wk_sb[:].rearrange("p (c m) -> p c m", m=GM),
            wk.rearrange("(c p) m -> p c m", p=128),
        )

        # ---- attention + projections, software-pipelined ----------------
        with tc.tile_pool(name="s_psum", bufs=2, space="PSUM") as s_psum, \
             tc.tile_pool(name="slab", bufs=PV_LAG + 1) as slab_pool, \
             tc.tile_pool(name="norm", bufs=1) as norm_pool, \
             tc.tile_pool(name="bcast", bufs=2) as bc_pool, \
             tc.tile_pool(name="shift", bufs=1) as shift_pool, \
             tc.tile_pool(name="dscr", bufs=2, space="DRAM") as dram_pool:

            def qk_quarter(t, w_sb, dst, qq):
                """Generator: Q or K projection m-tile t, q-quarter qq (512).
                Yields after small matmul groups so callers can interleave."""
                ps = qk_psum.tile([128, 512], F32, tag="qkp")
                q0 = qq * 512
                for dc in range(NDC):
                    nc.tensor.matmul(
                        ps[:],
                        w_sb[:, dc * GM + t * 128 : dc * GM + (t + 1) * 128],
                        xt[:, dc * S + q0 : dc * S + q0 + 512],
                        start=(dc == 0),
                        stop=(dc == NDC - 1),
                    )
                    if dc % 2 == 1:
                        yield
                if dst is qt8:
                    nc.scalar.activation(
                        dst[:, t, q0 : q0 + 512], ps[:], AF.Identity,
                        bias=bq_sb[:, t : t + 1],
                    )
                else:
                    nc.scalar.activation(dst[:, t, q0 : q0 + 512], ps[:], AF.Copy)
                yield

            def v_chunk(si, v_psum):
                """Generator: V projection keys-chunk si -> v8 (fp8)."""
                ps = v_psum.tile([128, GM], F32, tag="vp")
                for dc in range(NDC):
                    nc.tensor.matmul(
                        ps[:],
                        xt[:, dc * S + si * 128 : dc * S + (si + 1) * 128],
                        wv_sb[:, dc * GM : (dc + 1) * GM],
                        start=(dc == 0),
                        stop=(dc == NDC - 1),
                    )
                    if dc % 4 == 3:
                        yield
                nc.vector.tensor_copy(
                    v8[:, si // 2, si % 2, :, 0:HD],
                    ps[:].rearrange("p (h d) -> p h d", h=GH),
                )
                yield

            # jobs: j = 2*h + qh
            slabs = [None] * (2 * GH)

            s_pools = [s_psum]
            chunk_ctr = [0]

            def issue_job_scores(j, fillers, pattern, inject=None):
                """Scores+exp for job j, pulling filler matmul groups from
                `fillers` (a list of active generators) between chunks.
                `inject` maps chunk index -> thunk issued at that point."""
                h, qh = j // 2, j % 2
                slot, p0 = h // 2, 64 * (h % 2)
                sl = slab_pool.tile([128, NKC // 2, 2, JW], FP8, tag="slab")
                slabs[j] = sl
                psl = slice(p0, p0 + 64)
                for kc in range(NKC):
                    if inject and kc in inject:
                        inject[kc]()
                    u, jj = kc // 2, kc % 2
                    # kt8/qt8 [64, 1, N] viewed as a stride-0 [64, 2, N]
                    lhsT = kt8[psl, slot, kc * 128 : (kc + 1) * 128] \
                        .unsqueeze(1).broadcast_to([64, 2, 128])
                    sp = s_pools[chunk_ctr[0] % len(s_pools)].tile(
                        [128, JW], F32, tag="sp"
                    )
                    chunk_ctr[0] += 1
                    for qq in range(JW // 512):
                        q0 = qh * JW + qq * 512
                        rhs = qt8[psl, slot, q0 : q0 + 512] \
                            .unsqueeze(1).broadcast_to([64, 2, 512])
                        nc.tensor.matmul(
                            sp[:, qq * 512 : (qq + 1) * 512],
                            lhsT,
                            rhs,
                            start=True,
                            stop=True,
                            perf_mode=DR,
                        )
                    dst = sl[:, u, jj, :]
                    if pattern[kc] == "A":
                        nc.scalar.activation(dst, sp[:], AF.Exp, scale=EXP_SCALE)
                    else:
                        nc.vector.tensor_scalar(
                            dst.bitcast(I8), sp[:],
                            SCH_MUL, SCH_ADD, ALU.mult, ALU.add,
                        )
                    # pull ~1.4 filler steps per chunk so the 12 QK halves
                    # (slots 1-3) finish by the end of job 4
                    fillers_budget[0] += 1.4
                    while fillers and fillers_budget[0] >= 1.0:
                        try:
                            next(fillers[0])
                            fillers_budget[0] -= 1.0
                        except StopIteration:
                            fillers.pop(0)

            pending = {}  # j -> (po tile, bc tile)

            def issue_pv_den(j, o_psum):
                """PV matmuls + reciprocal + the den-broadcast DMA roundtrip.
                The otp multiply is deferred (issue_norm_mult) so the DVE
                never stalls in-order on the broadcast DMA."""
                h = j // 2
                sl = slabs[j]
                po = o_psum.tile([128, JW], F32, tag="op")
                for u in range(NKC // 2):
                    lhsT = v8[:, u, :, h, 0:VW]
                    for qs in range(JW // 512):
                        nc.tensor.matmul(
                            po[0:VW, qs * 512 : (qs + 1) * 512],
                            lhsT,
                            sl[:, u, :, qs * 512 : (qs + 1) * 512],
                            start=(u == 0),
                            stop=(u == NKC // 2 - 1),
                            perf_mode=DR,
                        )
                nrm = norm_pool.tile([VW, JW], F32, tag="nrm")
                nc.vector.reciprocal(nrm[HD:VW, :], po[HD:VW, :])
                scr = dram_pool.tile([JW], F32, tag="scr")
                nc.sync.dma_start(scr.unsqueeze(0), nrm[HD:VW, :])
                bc = bc_pool.tile([HD, JW], F32, tag="bc")
                nc.sync.dma_start(bc[:], scr.unsqueeze(0).broadcast_to([HD, JW]))
                pending[j] = (po, bc)
                slabs[j] = None
                issue_norm_mult(j)

            def issue_norm_mult(j):
                if j not in pending:
                    return
                po, bc = pending.pop(j)
                h, qh = j // 2, j % 2
                t = h // 2
                qsl = slice(qh * JW, (qh + 1) * JW)
                if h % 2 == 0:
                    nc.vector.tensor_mul(otp[t][0:HD, qsl], po[0:HD, :], bc[:])
                else:
                    tmp = shift_pool.tile([HD, JW], BF16, tag="tmp")
                    nc.vector.tensor_mul(tmp[:], po[0:HD, :], bc[:])
                    nc.sync.dma_start(otp[t][HD:128, qsl], tmp[:])

            # -- prefix: QK slot 0, then V (PE-dense, exp engines idle) --
            qk_stack = ExitStack()
            qk_psum = qk_stack.enter_context(
                tc.tile_pool(name="qk_psum", bufs=2, space="PSUM")
            )
            for w_sb, dst in ((wq_sb, qt8), (wk_sb, kt8)):
                for qq in range(4):
                    for _ in qk_quarter(0, w_sb, dst, qq):
                        pass
            v_stack = ExitStack()
            v_psum = v_stack.enter_context(
                tc.tile_pool(name="v_psum", bufs=2, space="PSUM")
            )
            for si in range(NKC):
                for _ in v_chunk(si, v_psum):
                    pass
            v_stack.close()

            # -- pipelined jobs --
            # fillers: QK slots 1..3 emitted between score chunks of jobs 0..3
            fillers = []
            fillers_budget = [0.0]
            for t in range(1, GH // 2):
                for w_sb, dst in ((wq_sb, qt8), (wk_sb, kt8)):
                    for qq in range(4):
                        fillers.append(qk_quarter(t, w_sb, dst, qq))

            o_stack = ExitStack()
            o_psum = None
            for j in range(2 * GH):
                if j < 5:
                    pattern = _PAT_EARLY
                else:
                    pattern = _PAT_STEADY
                issue_job_scores(j, fillers, pattern)
                if j == PV_LAG - 1:
                    # drain remaining QK fillers; free their psum for PV
                    while fillers:
                        try:
                            next(fillers[0])
                        except StopIteration:
                            fillers.pop(0)
                    qk_stack.close()
                    o_psum = o_stack.enter_context(
                        tc.tile_pool(name="o_psum", bufs=1, space="PSUM")
                    )
                    s_x_stack = ExitStack()
                    s_x = s_x_stack.enter_context(
                        tc.tile_pool(name="s_x", bufs=1, space="PSUM")
                    )
                    s_pools.extend([s_psum, s_x])
                if j >= PV_LAG:
                    issue_pv_den(j - PV_LAG, o_psum)
            # trailing PVs: reuse s_x's banks as a second PV buffer so the
            # per-job den/mult chains overlap. The output projection runs
            # interleaved, borrowing idle scores-psum tiles: per (ec, qh)
            # half, the mt0-2 accumulation pre-runs during the trailing
            # chains and only the mt3 matmuls wait on the final norm.
            s_x_stack.close()
            o2_stack = ExitStack()
            o2 = o2_stack.enter_context(
                tc.tile_pool(name="o2_psum", bufs=1, space="PSUM")
            )
            y_pool = o2_stack.enter_context(tc.tile_pool(name="y", bufs=2))

            out_tiles = {}

            def out_prework(i):
                ec, qh = i // 2, i % 2
                ps = s_psum.tile([128, JW], F32, tag="sp")
                out_tiles[i] = ps
                for mt in range(GM // 128 - 1):
                    lhsT = wo_sb[:, mt * D + ec * 128 : mt * D + (ec + 1) * 128]
                    for qb in range(2):
                        q0 = qh * JW + qb * 512
                        nc.tensor.matmul(
                            ps[:, qb * 512 : (qb + 1) * 512],
                            lhsT,
                            otp[mt][:, q0 : q0 + 512],
                            start=(mt == 0),
                            stop=False,
                        )

            def out_finish(i):
                ec, qh = i // 2, i % 2
                ps = out_tiles.pop(i)
                mt = GM // 128 - 1
                lhsT = wo_sb[:, mt * D + ec * 128 : mt * D + (ec + 1) * 128]
                for qb in range(2):
                    q0 = qh * JW + qb * 512
                    nc.tensor.matmul(
                        ps[:, qb * 512 : (qb + 1) * 512],
                        lhsT,
                        otp[mt][:, q0 : q0 + 512],
                        start=False,
                        stop=True,
                    )
                y_sb = y_pool.tile([128, JW], F32, tag="y")
                nc.scalar.activation(
                    y_sb[:], ps[:], AF.Identity, bias=bo_sb[:, ec : ec + 1]
                )
                eng = nc.sync if i % 2 == 0 else nc.scalar
                eng.dma_start(
                    outT[ec * 128 : (ec + 1) * 128, qh * JW : (qh + 1) * JW],
                    y_sb[:],
                )

            for i, j in enumerate(range(2 * GH - PV_LAG, 2 * GH)):
                issue_pv_den(j, o2 if i % 2 == 0 else o_psum)
                issue_norm_mult(j - 1)
                if i == 2:
                    out_prework(0)
                if i == 3:
                    out_prework(1)
            issue_norm_mult(2 * GH - 1)
            for i in range(2 * NDC):
                out_finish(i)
                if i + 2 < 2 * NDC:
                    out_prework(i + 2)
            o2_stack.close()
            o_stack.close()
        phase1.close()

    return nc


_NC = None
_last_in_maps = None


def _get_program():
    global _NC
    if _NC is None:
        _NC = _build_program()
    return _NC


def make_in_maps(x, Wq, bq, Wk, bk, Wv, bv, Wo, bo):
    x = np.asarray(x, np.float32)
    bf = ml_dtypes.bfloat16
    in_maps = []
    for c in range(8):
        b, g = c // 2, c % 2
        sl = slice(g * GM, (g + 1) * GM)
        wo_slice = np.asarray(Wo, np.float32)[:, sl].T  # [512, 1024]
        # fold bv and half of bo into the output bias
        bo_eff = np.asarray(bo, np.float32) / 2.0 + np.asarray(bv, np.float32)[sl] @ wo_slice
        in_maps.append(
            {
                "xT": np.ascontiguousarray(x[b].T).astype(bf),
                "wq": np.ascontiguousarray(np.asarray(Wq, np.float32)[sl, :].T).astype(bf),
                "wk": np.ascontiguousarray(np.asarray(Wk, np.float32)[sl, :].T).astype(bf),
                "wv": np.ascontiguousarray(np.asarray(Wv, np.float32)[sl, :].T).astype(bf),
                "wo": np.ascontiguousarray(
                    wo_slice.reshape(GM // 128, 128, D).transpose(1, 0, 2).reshape(128, (GM // 128) * D)
                ).astype(bf),
                "bq": np.ascontiguousarray(np.asarray(bq, np.float32)[sl]),
                "bo": np.ascontiguousarray(bo_eff.astype(np.float32)),
            }
        )
    return in_maps


def expected_partial(c, x, Wq, bq, Wk, bk, Wv, bv, Wo, bo):
    """Numpy recomputation of core c's partial outT [D, S] (f32)."""
    b, g = c // 2, c % 2
    sl = slice(g * GM, (g + 1) * GM)
    xb = np.asarray(x, np.float32)[b]  # [S, D]
    Q = xb @ np.asarray(Wq, np.float32)[sl, :].T + np.asarray(bq, np.float32)[sl]
    K_ = xb @ np.asarray(Wk, np.float32)[sl, :].T + np.asarray(bk, np.float32)[sl]
    V = xb @ np.asarray(Wv, np.float32)[sl, :].T + np.asarray(bv, np.float32)[sl]
    out = np.empty((S, GM), np.float32)
    for h in range(GH):
        hs = slice(h * HD, (h + 1) * HD)
        sc = Q[:, hs] @ K_[:, hs].T / np.sqrt(HD)
        e = np.exp(sc - sc.max(-1, keepdims=True))
        out[:, hs] = (e @ V[:, hs]) / e.sum(-1, keepdims=True)
    y = out @ np.asarray(Wo, np.float32)[:, sl].T + np.asarray(bo, np.float32) / 2.0
    return np.ascontiguousarray(y.T)  # [D, S]


def kernel(x, Wq, bq, Wk, bk, Wv, bv, Wo, bo):
    in_maps = make_in_maps(x, Wq, bq, Wk, bk, Wv, bv, Wo, bo)
    global _last_in_maps
    _last_in_maps = in_maps
    nc = _get_program()
    res = run_bass_kernel_spmd(nc, in_maps, core_ids=list(range(8)))
    out = np.empty((B, S, D), np.float32)
    for b in range(B):
        acc = res.results[2 * b]["outT"].astype(np.float32) + res.results[
            2 * b + 1
        ]["outT"].astype(np.float32)
        out[b] = acc.T
    return out


# revision 85
# speedup vs baseline: 1.0431x; 1.0230x over previous
"""Multi-head self-attention (B=4, S=2048, D=1024, H=16, Hd=64) on 8 TRN2 cores.

Sharding: core c -> (batch b = c//2, head-group g = c%2 of 8 heads).
Each core computes its batch's 8 heads end-to-end plus the partial output
projection for its head group; the host sums the two head-group partials
per batch. No collectives.

Device layout is fully transposed: activations are [feature(partitions),
seq(free)]. QKV and output projections run in bf16. The attention part
(scores QK^T and PV) runs in fp8e4m3 with DoubleRow perf mode at 0.5
cycles/row. For scores, the two DoubleRow k-subtiles read the SAME 64-dim
Q/K data via a stride-0 middle dim (computing 2*QK^T); the doubling is
folded into the exp scale (1/16 instead of 1/8). V is stored as
[128 keys x kc-pair x 8 head x 65] with an all-ones 65th column producing
the softmax denominator inside the PV psum.

exp() splits across the Activation engine (exact exp, fp8 output) and the
DVE (Schraudolph: fp8e4m3 bits = trunc(s*scale*8/ln2 + 56.05) via one
tensor_scalar mult+add writing int8). Attention is pipelined as 16 jobs
(head, q-half) with PV lagging 4 jobs; Q/K projection m-tiles 1..3 are
emitted piecewise between score chunks so the PE fills exp-bound gaps.
K's bias is dropped (softmax-invariant); V's bias is folded into the
output-projection bias on the host.
"""

from contextlib import ExitStack

import numpy as np
import ml_dtypes

import concourse.bass as bass
import concourse.tile as tile
from concourse import mybir
from concourse.bass_utils import run_bass_kernel_spmd
from concourse.vector_clock import ScopedClock
from bass_rust import InstNoOp, SyncInfo

BF16 = mybir.dt.bfloat16
F32 = mybir.dt.float32
FP8 = mybir.dt.float8e4
I8 = mybir.dt.int8
AF = mybir.ActivationFunctionType
ALU = mybir.AluOpType
DR = mybir.MatmulPerfMode.DoubleRow

B, S, D = 4, 2048, 1024
H, HD = 16, 64
GH = 8          # heads per core (head-group size)
GM = GH * HD    # 512 head dims per core
NDC = 8         # d chunks of 128 (contraction for projections)
NKC = 16        # k chunks of 128
VW = HD + 1     # 65: per-head V columns + ones column
VWP = HD + 2    # 66: v8 stores heads 2-byte aligned (dual-fp8 LW requires
                # even weight byte offsets; 65-wide heads would give odd ones)
JW = S // 2     # 1024: q-width of one attention job
PV_LAG = 4      # jobs between scores(j) and PV(j)

# scores psum holds 2*QK^T (stride-0 DoubleRow pair), so exp scale is 1/16.
EXP_SCALE = 0.0625
SCH_MUL = EXP_SCALE * 8.0 / float(np.log(2.0))
SCH_ADD = 56.05


def _mk_pattern(counts):
    accs = {k: 0.0 for k in counts}
    total = sum(counts.values())
    out = []
    for _ in range(total):
        for k in counts:
            accs[k] += counts[k] / total
        k = max(accs, key=lambda q: accs[q])
        accs[k] -= 1.0
        out.append(k)
    return out


# exp engine per (kc) chunk within a job: 'A' = Act exact, 'D' = DVE
# Schraudolph. Early jobs carry QK copies on Act + V copies on DVE; later
# jobs carry norm work on DVE.
_PAT_EARLY = _mk_pattern({"A": 8, "D": 8})
_PAT_STEADY = _mk_pattern({"A": 11, "D": 5})
_PAT_STEADY2 = _mk_pattern({"A": 9, "D": 7})

_META_TYPES = ("TileBranchInst", "BassTileLoopBlock", "BassTilePoolBoundary")


class _TileCtx(tile.TileContext):
    """Splits multi-sem-wait instructions: the pinned walrus rejects any TPB
    instruction carrying more than one sem-wait, while Tile emits joins and a
    global end-of-context drain with several."""

    def _split_waits(self, ordered):
        nc = self.nc
        for bb_name, insts in ordered.items():
            out = []
            for inst in insts:
                si = inst.sync_info
                if (
                    si is not None
                    and si.on_wait
                    and len(si.on_wait) > 1
                    and type(inst).__name__ not in _META_TYPES
                    and inst.engine != mybir.EngineType.Unassigned
                ):
                    waits = list(si.on_wait)
                    for w in waits[:-1]:
                        nop = InstNoOp(
                            name=nc.get_next_instruction_name(), ins=[], outs=[]
                        )
                        nop.engine = inst.engine
                        nop.sync_info = SyncInfo(on_wait=[w], on_update=[])
                        out.append(nop)
                    inst.sync_info = SyncInfo(
                        on_wait=[waits[-1]], on_update=list(si.on_update)
                    )
                out.append(inst)
            ordered[bb_name] = out

    def _lower_ordered_insts(self, ordered):
        self._split_waits(ordered)
        super()._lower_ordered_insts(ordered)

    def _drain_and_barrier(self, tick_clock, wait_clock):
        drain_inst = self.nc.sync.drain()
        wait_clock.add_sem_waits(
            drain_inst.ins, ScopedClock({None: tick_clock.global_clock})
        )
        si = drain_inst.ins.sync_info
        waits = list(si.on_wait) if si is not None else []
        if len(waits) > 1:
            drain_inst.ins.sync_info = SyncInfo(
                on_wait=waits[:1], on_update=list(si.on_update)
            )
            for w in waits[1:]:
                extra = self.nc.sync.drain()
                extra.ins.sync_info = SyncInfo(on_wait=[w], on_update=[])

        self.nc.all_engine_barrier()
        assert self.sems is not None
        popped = self.nc._tile_sem_poison_stack.pop()
        assert popped is self._sem_poison
        self.nc.clear_and_free_semaphores(list(self.sems.allocated().values()))
        self.nc.all_engine_barrier()


def _build_program():
    nc = bass.Bass(trn_type="TRN2", debug=False, num_devices=8)

    xT = nc.dram_tensor("xT", [D, S], BF16, kind="ExternalInput").ap()
    wq = nc.dram_tensor("wq", [D, GM], BF16, kind="ExternalInput").ap()
    wk = nc.dram_tensor("wk", [D, GM], BF16, kind="ExternalInput").ap()
    wv = nc.dram_tensor("wv", [D, GM], BF16, kind="ExternalInput").ap()
    # pair-major-reordered Wo.T slice: [128, 4 pairs x 1024]
    wo = nc.dram_tensor("wo", [128, (GM // 128) * D], BF16, kind="ExternalInput").ap()
    bq = nc.dram_tensor("bq", [GM], F32, kind="ExternalInput").ap()
    bo = nc.dram_tensor("bo", [D], F32, kind="ExternalInput").ap()
    outT = nc.dram_tensor("outT", [D, S], BF16, kind="ExternalOutput").ap()

    with _TileCtx(nc) as tc, ExitStack() as ctx:
        const_pool = ctx.enter_context(tc.tile_pool(name="const", bufs=1))
        act_pool = ctx.enter_context(tc.tile_pool(name="acts", bufs=1))

        # ---- constants / weights / inputs -------------------------------
        bq_sb = const_pool.tile([128, GM // 128], F32, tag="bq")
        nc.sync.dma_start(bq_sb[:], bq.rearrange("(c p) -> p c", p=128))
        bo_sb = const_pool.tile([128, NDC], F32, tag="bo")
        nc.sync.dma_start(bo_sb[:], bo.rearrange("(c p) -> p c", p=128))
        wo_sb = const_pool.tile([128, (GM // 128) * D], BF16, tag="wo")
        nc.sync.dma_start(wo_sb[:], wo[:, :])

        # persistent activations. qt8/kt8: [128, slot t, S]; head h lives at
        # partitions [64*(h%2), +64) of slot h//2.
        qt8 = act_pool.tile([128, GH // 2, S], FP8, tag="qt8")
        kt8 = act_pool.tile([128, GH // 2, S], FP8, tag="kt8")
        v8 = act_pool.tile([128, NKC // 2, 2, GH, VWP], FP8, tag="v8")
        nc.gpsimd.memset(v8[:, :, :, :, HD:VWP], 1.0)
        otp = [
            act_pool.tile([128, S], BF16, name=f"otp{t}", tag=f"otp{t}")
            for t in range(GH // 2)
        ]

        phase1 = ExitStack()
        w_pool = phase1.enter_context(tc.tile_pool(name="wts", bufs=1))
        xt = w_pool.tile([128, NDC * S], BF16, tag="xt")
        for t in range(NDC):
            eng = nc.sync if t % 2 == 0 else nc.scalar
            eng.dma_start(
                xt[:, t * S : (t + 1) * S], xT[t * 128 : (t + 1) * 128, :]
            )
        wv_sb = w_pool.tile([128, NDC * GM], BF16, tag="wv")
        nc.scalar.dma_start(
            wv_sb[:].rearrange("p (c m) -> p c m", m=GM),
            wv.rearrange("(c p) m -> p c m", p=128),
        )
        wq_sb = w_pool.tile([128, NDC * GM], BF16, tag="wq")
        nc.sync.dma_start(
            wq_sb[:].rearrange("p (c m) -> p c m", m=GM),
            wq.rearrange("(c p) m -> p c m", p=128),
        )
        wk_sb = w_pool.tile([128, NDC * GM], BF16, tag="wk")
        nc.sync.dma_start(
            wk_sb[:].rearrange("p (c m) -> p c m", m=GM),
            wk.rearrange("(c p) m -> p c m", p=128),
        )

        # ---- attention + projections, software-pipelined ----------------
        with tc.tile_pool(name="s_psum", bufs=2, space="PSUM") as s_psum, \
             tc.tile_pool(name="slab", bufs=PV_LAG + 1) as slab_pool, \
             tc.tile_pool(name="norm", bufs=1) as norm_pool, \
             tc.tile_pool(name="bcast", bufs=1) as bc_pool, \
             tc.tile_pool(name="shift", bufs=1) as shift_pool, \
             tc.tile_pool(name="dscr", bufs=2, space="DRAM") as dram_pool:

            def qk_quarter(t, w_sb, dst, qq):
                """Generator: Q or K projection m-tile t, q-quarter qq (512).
                Yields after small matmul groups so callers can interleave."""
                ps = qk_psum.tile([128, 512], F32, tag="qkp")
                q0 = qq * 512
                for dc in range(NDC):
                    nc.tensor.matmul(
                        ps[:],
                        w_sb[:, dc * GM + t * 128 : dc * GM + (t + 1) * 128],
                        xt[:, dc * S + q0 : dc * S + q0 + 512],
                        start=(dc == 0),
                        stop=(dc == NDC - 1),
                    )
                    if dc % 2 == 1:
                        yield
                if dst is qt8:
                    nc.scalar.activation(
                        dst[:, t, q0 : q0 + 512], ps[:], AF.Identity,
                        bias=bq_sb[:, t : t + 1],
                    )
                else:
                    nc.scalar.activation(dst[:, t, q0 : q0 + 512], ps[:], AF.Copy)
                yield

            def v_chunk(si, v_psum):
                """Generator: V projection keys-chunk si -> v8 (fp8)."""
                ps = v_psum.tile([128, GM], F32, tag="vp")
                for dc in range(NDC):
                    nc.tensor.matmul(
                        ps[:],
                        xt[:, dc * S + si * 128 : dc * S + (si + 1) * 128],
                        wv_sb[:, dc * GM : (dc + 1) * GM],
                        start=(dc == 0),
                        stop=(dc == NDC - 1),
                    )
                    if dc % 4 == 3:
                        yield
                nc.vector.tensor_copy(
                    v8[:, si // 2, si % 2, :, 0:HD],
                    ps[:].rearrange("p (h d) -> p h d", h=GH),
                )
                yield

            # jobs: j = 2*h + qh
            slabs = [None] * (2 * GH)

            s_pools = [s_psum]
            chunk_ctr = [0]

            def issue_job_scores(j, fillers, pattern, inject=None):
                """Scores+exp for job j, pulling filler matmul groups from
                `fillers` (a list of active generators) between chunks.
                `inject` maps chunk index -> thunk issued at that point."""
                h, qh = j // 2, j % 2
                slot, p0 = h // 2, 64 * (h % 2)
                sl = slab_pool.tile([128, NKC // 2, 2, JW], FP8, tag="slab")
                slabs[j] = sl
                psl = slice(p0, p0 + 64)
                for kc in range(NKC):
                    if inject and kc in inject:
                        inject[kc]()
                    u, jj = kc // 2, kc % 2
                    # kt8/qt8 [64, 1, N] viewed as a stride-0 [64, 2, N]
                    lhsT = kt8[psl, slot, kc * 128 : (kc + 1) * 128] \
                        .unsqueeze(1).broadcast_to([64, 2, 128])
                    sp = s_pools[chunk_ctr[0] % len(s_pools)].tile(
                        [128, JW], F32, tag="sp"
                    )
                    chunk_ctr[0] += 1
                    for qq in range(JW // 512):
                        q0 = qh * JW + qq * 512
                        rhs = qt8[psl, slot, q0 : q0 + 512] \
                            .unsqueeze(1).broadcast_to([64, 2, 512])
                        nc.tensor.matmul(
                            sp[:, qq * 512 : (qq + 1) * 512],
                            lhsT,
                            rhs,
                            start=True,
                            stop=True,
                            perf_mode=DR,
                        )
                    dst = sl[:, u, jj, :]
                    if pattern[kc] == "A":
                        nc.scalar.activation(dst, sp[:], AF.Exp, scale=EXP_SCALE)
                    else:
                        nc.vector.tensor_scalar(
                            dst.bitcast(I8), sp[:],
                            SCH_MUL, SCH_ADD, ALU.mult, ALU.add,
                        )
                    # pull ~1.4 filler steps per chunk so the 12 QK halves
                    # (slots 1-3) finish by the end of job 4
                    fillers_budget[0] += 1.4
                    while fillers and fillers_budget[0] >= 1.0:
                        try:
                            next(fillers[0])
                            fillers_budget[0] -= 1.0
                        except StopIteration:
                            fillers.pop(0)

            pending = {}  # j -> (po tile, bc tile)

            def issue_pv_den(j, o_psum):
                """PV matmuls + reciprocal + the den-broadcast DMA roundtrip.
                The otp multiply is deferred (issue_norm_mult) so the DVE
                never stalls in-order on the broadcast DMA."""
                h = j // 2
                sl = slabs[j]
                po = o_psum.tile([128, JW], F32, tag="op")
                for u in range(NKC // 2):
                    lhsT = v8[:, u, :, h, 0:VW]
                    for qs in range(JW // 512):
                        nc.tensor.matmul(
                            po[0:VW, qs * 512 : (qs + 1) * 512],
                            lhsT,
                            sl[:, u, :, qs * 512 : (qs + 1) * 512],
                            start=(u == 0),
                            stop=(u == NKC // 2 - 1),
                            perf_mode=DR,
                        )
                nrm = norm_pool.tile([VW, JW], F32, tag="nrm")
                nc.vector.reciprocal(nrm[HD:VW, :], po[HD:VW, :])
                scr = dram_pool.tile([JW], F32, tag="scr")
                nc.sync.dma_start(scr.unsqueeze(0), nrm[HD:VW, :])
                bc = bc_pool.tile([HD, JW], F32, tag="bc")
                nc.sync.dma_start(bc[:], scr.unsqueeze(0).broadcast_to([HD, JW]))
                pending[j] = (po, bc)
                slabs[j] = None
                issue_norm_mult(j)

            def issue_norm_mult(j):
                if j not in pending:
                    return
                po, bc = pending.pop(j)
                h, qh = j // 2, j % 2
                t = h // 2
                qsl = slice(qh * JW, (qh + 1) * JW)
                if h % 2 == 0:
                    nc.vector.tensor_mul(otp[t][0:HD, qsl], po[0:HD, :], bc[:])
                else:
                    tmp = shift_pool.tile([HD, JW], BF16, tag="tmp")
                    nc.vector.tensor_mul(tmp[:], po[0:HD, :], bc[:])
                    nc.sync.dma_start(otp[t][HD:128, qsl], tmp[:])

            # -- prefix: QK slot 0, then V (PE-dense, exp engines idle) --
            qk_stack = ExitStack()
            qk_psum = qk_stack.enter_context(
                tc.tile_pool(name="qk_psum", bufs=2, space="PSUM")
            )
            for w_sb, dst in ((wq_sb, qt8), (wk_sb, kt8)):
                for qq in range(4):
                    for _ in qk_quarter(0, w_sb, dst, qq):
                        pass
            v_stack = ExitStack()
            v_psum = v_stack.enter_context(
                tc.tile_pool(name="v_psum", bufs=2, space="PSUM")
            )
            for si in range(NKC):
                for _ in v_chunk(si, v_psum):
                    pass
            v_stack.close()

            # -- pipelined jobs --
            # fillers: QK slots 1..3 emitted between score chunks of jobs 0..3
            fillers = []
            fillers_budget = [0.0]
            for t in range(1, GH // 2):
                for w_sb, dst in ((wq_sb, qt8), (wk_sb, kt8)):
                    for qq in range(4):
                        fillers.append(qk_quarter(t, w_sb, dst, qq))

            o_stack = ExitStack()
            o_psum = None
            for j in range(2 * GH):
                if j < 5:
                    pattern = _PAT_EARLY
                else:
                    pattern = _PAT_STEADY
                issue_job_scores(j, fillers, pattern)
                if j == PV_LAG - 1:
                    # drain remaining QK fillers; free their psum for PV
                    while fillers:
                        try:
                            next(fillers[0])
                        except StopIteration:
                            fillers.pop(0)
                    qk_stack.close()
                    o_psum = o_stack.enter_context(
                        tc.tile_pool(name="o_psum", bufs=1, space="PSUM")
                    )
                    s_x_stack = ExitStack()
                    s_x = s_x_stack.enter_context(
                        tc.tile_pool(name="s_x", bufs=1, space="PSUM")
                    )
                    s_pools.extend([s_psum, s_x])
                if j >= PV_LAG:
                    issue_pv_den(j - PV_LAG, o_psum)
            # trailing PVs: reuse s_x's banks as a second PV buffer so the
            # per-job den/mult chains overlap. The output projection runs
            # interleaved, borrowing idle scores-psum tiles: per (ec, qh)
            # half, the mt0-2 accumulation pre-runs during the trailing
            # chains and only the mt3 matmuls wait on the final norm.
            s_x_stack.close()
            o2_stack = ExitStack()
            o2 = o2_stack.enter_context(
                tc.tile_pool(name="o2_psum", bufs=1, space="PSUM")
            )
            y_pool = o2_stack.enter_context(tc.tile_pool(name="y", bufs=4))

            out_tiles = {}

            def out_prework(i):
                ec, qh = i // 2, i % 2
                ps = s_psum.tile([128, JW], F32, tag="sp")
                out_tiles[i] = ps
                for mt in range(GM // 128 - 1):
                    lhsT = wo_sb[:, mt * D + ec * 128 : mt * D + (ec + 1) * 128]
                    for qb in range(2):
                        q0 = qh * JW + qb * 512
                        nc.tensor.matmul(
                            ps[:, qb * 512 : (qb + 1) * 512],
                            lhsT,
                            otp[mt][:, q0 : q0 + 512],
                            start=(mt == 0),
                            stop=False,
                        )

            def out_finish(i):
                ec, qh = i // 2, i % 2
                ps = out_tiles.pop(i)
                mt = GM // 128 - 1
                lhsT = wo_sb[:, mt * D + ec * 128 : mt * D + (ec + 1) * 128]
                for qb in range(2):
                    q0 = qh * JW + qb * 512
                    nc.tensor.matmul(
                        ps[:, qb * 512 : (qb + 1) * 512],
                        lhsT,
                        otp[mt][:, q0 : q0 + 512],
                        start=False,
                        stop=True,
                    )
                y_sb = y_pool.tile([128, JW], BF16, tag="y")
                nc.scalar.activation(
                    y_sb[:], ps[:], AF.Identity, bias=bo_sb[:, ec : ec + 1]
                )
                eng = nc.sync if i % 2 == 0 else nc.scalar
                eng.dma_start(
                    outT[ec * 128 : (ec + 1) * 128, qh * JW : (qh + 1) * JW],
                    y_sb[:],
                )

            for i, j in enumerate(range(2 * GH - PV_LAG, 2 * GH)):
                issue_pv_den(j, o2 if i % 2 == 0 else o_psum)
                issue_norm_mult(j - 1)
                if i == 2:
                    out_prework(0)
                if i == 3:
                    out_prework(1)
            issue_norm_mult(2 * GH - 1)
            for i in range(2 * NDC):
                out_finish(i)
                if i + 2 < 2 * NDC:
                    out_prework(i + 2)
            o2_stack.close()
            o_stack.close()
        phase1.close()

    return nc


_NC = None
_last_in_maps = None


def _get_program():
    global _NC
    if _NC is None:
        _NC = _build_program()
    return _NC


def make_in_maps(x, Wq, bq, Wk, bk, Wv, bv, Wo, bo):
    x = np.asarray(x, np.float32)
    bf = ml_dtypes.bfloat16
    in_maps = []
    for c in range(8):
        b, g = c // 2, c % 2
        sl = slice(g * GM, (g + 1) * GM)
        wo_slice = np.asarray(Wo, np.float32)[:, sl].T  # [512, 1024]
        # fold bv and half of bo into the output bias
        bo_eff = np.asarray(bo, np.float32) / 2.0 + np.asarray(bv, np.float32)[sl] @ wo_slice
        in_maps.append(
            {
                "xT": np.ascontiguousarray(x[b].T).astype(bf),
                "wq": np.ascontiguousarray(np.asarray(Wq, np.float32)[sl, :].T).astype(bf),
                "wk": np.ascontiguousarray(np.asarray(Wk, np.float32)[sl, :].T).astype(bf),
                "wv": np.ascontiguousarray(np.asarray(Wv, np.float32)[sl, :].T).astype(bf),
                "wo": np.ascontiguousarray(
                    wo_slice.reshape(GM // 128, 128, D).transpose(1, 0, 2).reshape(128, (GM // 128) * D)
                ).astype(bf),
                "bq": np.ascontiguousarray(np.asarray(bq, np.float32)[sl]),
                "bo": np.ascontiguousarray(bo_eff.astype(np.float32)),
            }
        )
    return in_maps


def expected_partial(c, x, Wq, bq, Wk, bk, Wv, bv, Wo, bo):
    """Numpy recomputation of core c's partial outT [D, S] (f32)."""
    b, g = c // 2, c % 2
    sl = slice(g * GM, (g + 1) * GM)
    xb = np.asarray(x, np.float32)[b]  # [S, D]
    Q = xb @ np.asarray(Wq, np.float32)[sl, :].T + np.asarray(bq, np.float32)[sl]
    K_ = xb @ np.asarray(Wk, np.float32)[sl, :].T + np.asarray(bk, np.float32)[sl]
    V = xb @ np.asarray(Wv, np.float32)[sl, :].T + np.asarray(bv, np.float32)[sl]
    out = np.empty((S, GM), np.float32)
    for h in range(GH):
        hs = slice(h * HD, (h + 1) * HD)
        sc = Q[:, hs] @ K_[:, hs].T / np.sqrt(HD)
        e = np.exp(sc - sc.max(-1, keepdims=True))
        out[:, hs] = (e @ V[:, hs]) / e.sum(-1, keepdims=True)
    y = out @ np.asarray(Wo, np.float32)[:, sl].T + np.asarray(bo, np.float32) / 2.0
    return np.ascontiguousarray(y.T)  # [D, S]


def kernel(x, Wq, bq, Wk, bk, Wv, bv, Wo, bo):
    in_maps = make_in_maps(x, Wq, bq, Wk, bk, Wv, bv, Wo, bo)
    global _last_in_maps
    _last_in_maps = in_maps
    nc = _get_program()
    res = run_bass_kernel_spmd(nc, in_maps, core_ids=list(range(8)))
    out = np.empty((B, S, D), np.float32)
    for b in range(B):
        acc = res.results[2 * b]["outT"].astype(np.float32) + res.results[
            2 * b + 1
        ]["outT"].astype(np.float32)
        out[b] = acc.T
    return out


# revision 92
# speedup vs baseline: 1.0433x; 1.0002x over previous
"""Multi-head self-attention (B=4, S=2048, D=1024, H=16, Hd=64) on 8 TRN2 cores.

Sharding: core c -> (batch b = c//2, head-group g = c%2 of 8 heads).
Each core computes its batch's 8 heads end-to-end plus the partial output
projection for its head group; the host sums the two head-group partials
per batch. No collectives.

Device layout is fully transposed: activations are [feature(partitions),
seq(free)]. QKV and output projections run in bf16. The attention part
(scores QK^T and PV) runs in fp8e4m3 with DoubleRow perf mode at 0.5
cycles/row. For scores, the two DoubleRow k-subtiles read the SAME 64-dim
Q/K data via a stride-0 middle dim (computing 2*QK^T); the doubling is
folded into the exp scale (1/16 instead of 1/8). V is stored as
[128 keys x kc-pair x 8 head x 65] with an all-ones 65th column producing
the softmax denominator inside the PV psum.

exp() splits across the Activation engine (exact exp, fp8 output) and the
DVE (Schraudolph: fp8e4m3 bits = trunc(s*scale*8/ln2 + 56.05) via one
tensor_scalar mult+add writing int8). Attention is pipelined as 16 jobs
(head, q-half) with PV lagging 4 jobs; Q/K projection m-tiles 1..3 are
emitted piecewise between score chunks so the PE fills exp-bound gaps.
K's bias is dropped (softmax-invariant); V's bias is folded into the
output-projection bias on the host.
"""

from contextlib import ExitStack

import numpy as np
import ml_dtypes

import concourse.bass as bass
import concourse.tile as tile
from concourse import mybir
from concourse.bass_utils import run_bass_kernel_spmd
from concourse.vector_clock import ScopedClock
from bass_rust import InstNoOp, SyncInfo

BF16 = mybir.dt.bfloat16
F32 = mybir.dt.float32
FP8 = mybir.dt.float8e4
I8 = mybir.dt.int8
AF = mybir.ActivationFunctionType
ALU = mybir.AluOpType
DR = mybir.MatmulPerfMode.DoubleRow

B, S, D = 4, 2048, 1024
H, HD = 16, 64
GH = 8          # heads per core (head-group size)
GM = GH * HD    # 512 head dims per core
NDC = 8         # d chunks of 128 (contraction for projections)
NKC = 16        # k chunks of 128
VW = HD + 1     # 65: per-head V columns + ones column
VWP = HD + 2    # 66: v8 stores heads 2-byte aligned (dual-fp8 LW requires
                # even weight byte offsets; 65-wide heads would give odd ones)
JW = S // 2     # 1024: q-width of one attention job
PV_LAG = 4      # jobs between scores(j) and PV(j)

# scores psum holds 2*QK^T (stride-0 DoubleRow pair), so exp scale is 1/16.
EXP_SCALE = 0.0625
SCH_MUL = EXP_SCALE * 8.0 / float(np.log(2.0))
SCH_ADD = 56.05


def _mk_pattern(counts):
    accs = {k: 0.0 for k in counts}
    total = sum(counts.values())
    out = []
    for _ in range(total):
        for k in counts:
            accs[k] += counts[k] / total
        k = max(accs, key=lambda q: accs[q])
        accs[k] -= 1.0
        out.append(k)
    return out


# exp engine per (kc) chunk within a job: 'A' = Act exact, 'D' = DVE
# Schraudolph. Early jobs carry QK copies on Act + V copies on DVE; later
# jobs carry norm work on DVE.
_PAT_EARLY = _mk_pattern({"A": 8, "D": 8})
_PAT_STEADY = _mk_pattern({"A": 11, "D": 5})
_PAT_STEADY2 = _mk_pattern({"A": 9, "D": 7})

_META_TYPES = ("TileBranchInst", "BassTileLoopBlock", "BassTilePoolBoundary")


class _TileCtx(tile.TileContext):
    """Splits multi-sem-wait instructions: the pinned walrus rejects any TPB
    instruction carrying more than one sem-wait, while Tile emits joins and a
    global end-of-context drain with several."""

    def _split_waits(self, ordered):
        nc = self.nc
        for bb_name, insts in ordered.items():
            out = []
            for inst in insts:
                si = inst.sync_info
                if (
                    si is not None
                    and si.on_wait
                    and len(si.on_wait) > 1
                    and type(inst).__name__ not in _META_TYPES
                    and inst.engine != mybir.EngineType.Unassigned
                ):
                    waits = list(si.on_wait)
                    for w in waits[:-1]:
                        nop = InstNoOp(
                            name=nc.get_next_instruction_name(), ins=[], outs=[]
                        )
                        nop.engine = inst.engine
                        nop.sync_info = SyncInfo(on_wait=[w], on_update=[])
                        out.append(nop)
                    inst.sync_info = SyncInfo(
                        on_wait=[waits[-1]], on_update=list(si.on_update)
                    )
                out.append(inst)
            ordered[bb_name] = out

    def _lower_ordered_insts(self, ordered):
        self._split_waits(ordered)
        super()._lower_ordered_insts(ordered)

    def _drain_and_barrier(self, tick_clock, wait_clock):
        drain_inst = self.nc.sync.drain()
        wait_clock.add_sem_waits(
            drain_inst.ins, ScopedClock({None: tick_clock.global_clock})
        )
        si = drain_inst.ins.sync_info
        waits = list(si.on_wait) if si is not None else []
        if len(waits) > 1:
            drain_inst.ins.sync_info = SyncInfo(
                on_wait=waits[:1], on_update=list(si.on_update)
            )
            for w in waits[1:]:
                extra = self.nc.sync.drain()
                extra.ins.sync_info = SyncInfo(on_wait=[w], on_update=[])

        self.nc.all_engine_barrier()
        assert self.sems is not None
        popped = self.nc._tile_sem_poison_stack.pop()
        assert popped is self._sem_poison
        self.nc.clear_and_free_semaphores(list(self.sems.allocated().values()))
        self.nc.all_engine_barrier()


def _build_program():
    nc = bass.Bass(trn_type="TRN2", debug=False, num_devices=8)

    xT = nc.dram_tensor("xT", [D, S], BF16, kind="ExternalInput").ap()
    wq = nc.dram_tensor("wq", [D, GM], BF16, kind="ExternalInput").ap()
    wk = nc.dram_tensor("wk", [D, GM], BF16, kind="ExternalInput").ap()
    wv = nc.dram_tensor("wv", [D, GM], BF16, kind="ExternalInput").ap()
    # pair-major-reordered Wo.T slice: [128, 4 pairs x 1024]
    wo = nc.dram_tensor("wo", [128, (GM // 128) * D], BF16, kind="ExternalInput").ap()
    bq = nc.dram_tensor("bq", [GM], F32, kind="ExternalInput").ap()
    bo = nc.dram_tensor("bo", [D], F32, kind="ExternalInput").ap()
    outT = nc.dram_tensor("outT", [D, S], BF16, kind="ExternalOutput").ap()

    with _TileCtx(nc) as tc, ExitStack() as ctx:
        const_pool = ctx.enter_context(tc.tile_pool(name="const", bufs=1))
        act_pool = ctx.enter_context(tc.tile_pool(name="acts", bufs=1))

        # ---- constants / weights / inputs -------------------------------
        bq_sb = const_pool.tile([128, GM // 128], F32, tag="bq")
        nc.sync.dma_start(bq_sb[:], bq.rearrange("(c p) -> p c", p=128))
        bo_sb = const_pool.tile([128, NDC], F32, tag="bo")
        nc.sync.dma_start(bo_sb[:], bo.rearrange("(c p) -> p c", p=128))
        wo_sb = const_pool.tile([128, (GM // 128) * D], BF16, tag="wo")
        nc.sync.dma_start(wo_sb[:], wo[:, :])

        # persistent activations. qt8/kt8: [128, slot t, S]; head h lives at
        # partitions [64*(h%2), +64) of slot h//2.
        qt8 = act_pool.tile([128, GH // 2, S], FP8, tag="qt8")
        kt8 = act_pool.tile([128, GH // 2, S], FP8, tag="kt8")
        v8 = act_pool.tile([128, NKC // 2, 2, GH, VWP], FP8, tag="v8")
        nc.gpsimd.memset(v8[:, :, :, :, HD:VWP], 1.0)
        otp = [
            act_pool.tile([128, S], BF16, name=f"otp{t}", tag=f"otp{t}")
            for t in range(GH // 2)
        ]

        phase1 = ExitStack()
        w_pool = phase1.enter_context(tc.tile_pool(name="wts", bufs=1))
        xt = w_pool.tile([128, NDC * S], BF16, tag="xt")
        for t in range(NDC):
            eng = nc.sync if t % 2 == 0 else nc.scalar
            eng.dma_start(
                xt[:, t * S : (t + 1) * S], xT[t * 128 : (t + 1) * 128, :]
            )
        wv_sb = w_pool.tile([128, NDC * GM], BF16, tag="wv")
        nc.scalar.dma_start(
            wv_sb[:].rearrange("p (c m) -> p c m", m=GM),
            wv.rearrange("(c p) m -> p c m", p=128),
        )
        wq_sb = w_pool.tile([128, NDC * GM], BF16, tag="wq")
        nc.sync.dma_start(
            wq_sb[:].rearrange("p (c m) -> p c m", m=GM),
            wq.rearrange("(c p) m -> p c m", p=128),
        )
        wk_sb = w_pool.tile([128, NDC * GM], BF16, tag="wk")
        nc.sync.dma_start(
            wk_sb[:].rearrange("p (c m) -> p c m", m=GM),
            wk.rearrange("(c p) m -> p c m", p=128),
        )

        # ---- attention + projections, software-pipelined ----------------
        with tc.tile_pool(name="s_psum", bufs=2, space="PSUM") as s_psum, \
             tc.tile_pool(name="slab", bufs=PV_LAG + 1) as slab_pool, \
             tc.tile_pool(name="norm", bufs=1) as norm_pool, \
             tc.tile_pool(name="bcast", bufs=2) as bc_pool, \
             tc.tile_pool(name="shift", bufs=1) as shift_pool, \
             tc.tile_pool(name="dscr", bufs=2, space="DRAM") as dram_pool:

            def qk_quarter(t, w_sb, dst, qq):
                """Generator: Q or K projection m-tile t, q-quarter qq (512).
                Yields after small matmul groups so callers can interleave."""
                ps = qk_psum.tile([128, 512], F32, tag="qkp")
                q0 = qq * 512
                for dc in range(NDC):
                    nc.tensor.matmul(
                        ps[:],
                        w_sb[:, dc * GM + t * 128 : dc * GM + (t + 1) * 128],
                        xt[:, dc * S + q0 : dc * S + q0 + 512],
                        start=(dc == 0),
                        stop=(dc == NDC - 1),
                    )
                    if dc % 2 == 1:
                        yield
                if dst is qt8:
                    nc.scalar.activation(
                        dst[:, t, q0 : q0 + 512], ps[:], AF.Identity,
                        bias=bq_sb[:, t : t + 1],
                    )
                else:
                    nc.scalar.activation(dst[:, t, q0 : q0 + 512], ps[:], AF.Copy)
                yield

            def v_chunk(si, v_psum):
                """Generator: V projection keys-chunk si -> v8 (fp8)."""
                ps = v_psum.tile([128, GM], F32, tag="vp")
                for dc in range(NDC):
                    nc.tensor.matmul(
                        ps[:],
                        xt[:, dc * S + si * 128 : dc * S + (si + 1) * 128],
                        wv_sb[:, dc * GM : (dc + 1) * GM],
                        start=(dc == 0),
                        stop=(dc == NDC - 1),
                    )
                    if dc % 4 == 3:
                        yield
                nc.vector.tensor_copy(
                    v8[:, si // 2, si % 2, :, 0:HD],
                    ps[:].rearrange("p (h d) -> p h d", h=GH),
                )
                yield

            # jobs: j = 2*h + qh
            slabs = [None] * (2 * GH)

            s_pools = [s_psum]
            chunk_ctr = [0]

            def issue_job_scores(j, fillers, pattern, inject=None):
                """Scores+exp for job j, pulling filler matmul groups from
                `fillers` (a list of active generators) between chunks.
                `inject` maps chunk index -> thunk issued at that point."""
                h, qh = j // 2, j % 2
                slot, p0 = h // 2, 64 * (h % 2)
                sl = slab_pool.tile([128, NKC // 2, 2, JW], FP8, tag="slab")
                slabs[j] = sl
                psl = slice(p0, p0 + 64)
                for kc in range(NKC):
                    if inject and kc in inject:
                        inject[kc]()
                    u, jj = kc // 2, kc % 2
                    # kt8/qt8 [64, 1, N] viewed as a stride-0 [64, 2, N]
                    lhsT = kt8[psl, slot, kc * 128 : (kc + 1) * 128] \
                        .unsqueeze(1).broadcast_to([64, 2, 128])
                    sp = s_pools[chunk_ctr[0] % len(s_pools)].tile(
                        [128, JW], F32, tag="sp"
                    )
                    chunk_ctr[0] += 1
                    for qq in range(JW // 512):
                        q0 = qh * JW + qq * 512
                        rhs = qt8[psl, slot, q0 : q0 + 512] \
                            .unsqueeze(1).broadcast_to([64, 2, 512])
                        nc.tensor.matmul(
                            sp[:, qq * 512 : (qq + 1) * 512],
                            lhsT,
                            rhs,
                            start=True,
                            stop=True,
                            perf_mode=DR,
                        )
                    dst = sl[:, u, jj, :]
                    if pattern[kc] == "A":
                        nc.scalar.activation(dst, sp[:], AF.Exp, scale=EXP_SCALE)
                    else:
                        nc.vector.tensor_scalar(
                            dst.bitcast(I8), sp[:],
                            SCH_MUL, SCH_ADD, ALU.mult, ALU.add,
                        )
                    # pull ~1.4 filler steps per chunk so the 12 QK halves
                    # (slots 1-3) finish by the end of job 4
                    fillers_budget[0] += 1.4
                    while fillers and fillers_budget[0] >= 1.0:
                        try:
                            next(fillers[0])
                            fillers_budget[0] -= 1.0
                        except StopIteration:
                            fillers.pop(0)

            pending = {}  # j -> (po tile, bc tile)

            def issue_pv_den(j, o_psum):
                """PV matmuls + reciprocal + the den-broadcast DMA roundtrip.
                The otp multiply is deferred (issue_norm_mult) so the DVE
                never stalls in-order on the broadcast DMA."""
                h = j // 2
                sl = slabs[j]
                po = o_psum.tile([128, JW], F32, tag="op")
                for u in range(NKC // 2):
                    lhsT = v8[:, u, :, h, 0:VW]
                    for qs in range(JW // 512):
                        nc.tensor.matmul(
                            po[0:VW, qs * 512 : (qs + 1) * 512],
                            lhsT,
                            sl[:, u, :, qs * 512 : (qs + 1) * 512],
                            start=(u == 0),
                            stop=(u == NKC // 2 - 1),
                            perf_mode=DR,
                        )
                nrm = norm_pool.tile([VW, JW], F32, tag="nrm")
                nc.vector.reciprocal(nrm[HD:VW, :], po[HD:VW, :])
                scr = dram_pool.tile([JW], F32, tag="scr")
                nc.sync.dma_start(scr.unsqueeze(0), nrm[HD:VW, :])
                bc = bc_pool.tile([HD, JW], F32, tag="bc")
                nc.sync.dma_start(bc[:], scr.unsqueeze(0).broadcast_to([HD, JW]))
                pending[j] = (po, bc)
                slabs[j] = None
                issue_norm_mult(j)

            def issue_norm_mult(j):
                if j not in pending:
                    return
                po, bc = pending.pop(j)
                h, qh = j // 2, j % 2
                t = h // 2
                qsl = slice(qh * JW, (qh + 1) * JW)
                if h % 2 == 0:
                    nc.vector.tensor_mul(otp[t][0:HD, qsl], po[0:HD, :], bc[:])
                else:
                    tmp = shift_pool.tile([HD, JW], BF16, tag="tmp")
                    nc.vector.tensor_mul(tmp[:], po[0:HD, :], bc[:])
                    nc.sync.dma_start(otp[t][HD:128, qsl], tmp[:])

            # -- prefix: QK slot 0, then V (PE-dense, exp engines idle) --
            qk_stack = ExitStack()
            qk_psum = qk_stack.enter_context(
                tc.tile_pool(name="qk_psum", bufs=2, space="PSUM")
            )
            for w_sb, dst in ((wq_sb, qt8), (wk_sb, kt8)):
                for qq in range(4):
                    for _ in qk_quarter(0, w_sb, dst, qq):
                        pass
            v_stack = ExitStack()
            v_psum = v_stack.enter_context(
                tc.tile_pool(name="v_psum", bufs=2, space="PSUM")
            )
            for si in range(NKC):
                for _ in v_chunk(si, v_psum):
                    pass
            v_stack.close()

            # -- pipelined jobs --
            # fillers: QK slots 1..3 emitted between score chunks of jobs 0..3
            fillers = []
            fillers_budget = [0.0]
            for t in range(1, GH // 2):
                for w_sb, dst in ((wq_sb, qt8), (wk_sb, kt8)):
                    for qq in range(4):
                        fillers.append(qk_quarter(t, w_sb, dst, qq))

            o_stack = ExitStack()
            o_psum = None
            for j in range(2 * GH):
                if j < 5:
                    pattern = _PAT_EARLY
                else:
                    pattern = _PAT_STEADY
                issue_job_scores(j, fillers, pattern)
                if j == PV_LAG - 1:
                    # drain remaining QK fillers; free their psum for PV
                    while fillers:
                        try:
                            next(fillers[0])
                        except StopIteration:
                            fillers.pop(0)
                    qk_stack.close()
                    o_psum = o_stack.enter_context(
                        tc.tile_pool(name="o_psum", bufs=1, space="PSUM")
                    )
                    s_x_stack = ExitStack()
                    s_x = s_x_stack.enter_context(
                        tc.tile_pool(name="s_x", bufs=1, space="PSUM")
                    )
                    s_pools.extend([s_psum, s_x])
                if j >= PV_LAG:
                    issue_pv_den(j - PV_LAG, o_psum)
            # trailing PVs: reuse s_x's banks as a second PV buffer so the
            # per-job den/mult chains overlap. The output projection runs
            # interleaved, borrowing idle scores-psum tiles: per (ec, qh)
            # half, the mt0-2 accumulation pre-runs during the trailing
            # chains and only the mt3 matmuls wait on the final norm.
            s_x_stack.close()
            o2_stack = ExitStack()
            o2 = o2_stack.enter_context(
                tc.tile_pool(name="o2_psum", bufs=1, space="PSUM")
            )
            y_pool = o2_stack.enter_context(tc.tile_pool(name="y", bufs=4))

            out_tiles = {}

            def out_prework(i):
                ec, qh = i // 2, i % 2
                ps = s_psum.tile([128, JW], F32, tag="sp")
                out_tiles[i] = ps
                for mt in range(GM // 128 - 1):
                    lhsT = wo_sb[:, mt * D + ec * 128 : mt * D + (ec + 1) * 128]
                    for qb in range(2):
                        q0 = qh * JW + qb * 512
                        nc.tensor.matmul(
                            ps[:, qb * 512 : (qb + 1) * 512],
                            lhsT,
                            otp[mt][:, q0 : q0 + 512],
                            start=(mt == 0),
                            stop=False,
                        )

            def out_finish(i):
                ec, qh = i // 2, i % 2
                ps = out_tiles.pop(i)
                mt = GM // 128 - 1
                lhsT = wo_sb[:, mt * D + ec * 128 : mt * D + (ec + 1) * 128]
                for qb in range(2):
                    q0 = qh * JW + qb * 512
                    nc.tensor.matmul(
                        ps[:, qb * 512 : (qb + 1) * 512],
                        lhsT,
                        otp[mt][:, q0 : q0 + 512],
                        start=False,
                        stop=True,
                    )
                y_sb = y_pool.tile([128, JW], BF16, tag="y")
                if i % 2 == 0:
                    nc.scalar.activation(
                        y_sb[:], ps[:], AF.Identity, bias=bo_sb[:, ec : ec + 1]
                    )
                else:
                    nc.vector.tensor_scalar_add(
                        y_sb[:], ps[:], bo_sb[:, ec : ec + 1]
                    )
                eng = nc.sync if i % 2 == 0 else nc.scalar
                eng.dma_start(
                    outT[ec * 128 : (ec + 1) * 128, qh * JW : (qh + 1) * JW],
                    y_sb[:],
                )

            for i, j in enumerate(range(2 * GH - PV_LAG, 2 * GH)):
                issue_pv_den(j, o2 if i % 2 == 0 else o_psum)
                issue_norm_mult(j - 1)
                if i == 2:
                    out_prework(0)
                if i == 3:
                    out_prework(1)
            issue_norm_mult(2 * GH - 1)
            for i in range(2 * NDC):
                out_finish(i)
                if i + 2 < 2 * NDC:
                    out_prework(i + 2)
            o2_stack.close()
            o_stack.close()
        phase1.close()

    return nc


_NC = None
_last_in_maps = None


def _get_program():
    global _NC
    if _NC is None:
        _NC = _build_program()
    return _NC


def make_in_maps(x, Wq, bq, Wk, bk, Wv, bv, Wo, bo):
    x = np.asarray(x, np.float32)
    bf = ml_dtypes.bfloat16
    in_maps = []
    for c in range(8):
        b, g = c // 2, c % 2
        sl = slice(g * GM, (g + 1) * GM)
        wo_slice = np.asarray(Wo, np.float32)[:, sl].T  # [512, 1024]
        # fold bv and half of bo into the output bias
        bo_eff = np.asarray(bo, np.float32) / 2.0 + np.asarray(bv, np.float32)[sl] @ wo_slice
        in_maps.append(
            {
                "xT": np.ascontiguousarray(x[b].T).astype(bf),
                "wq": np.ascontiguousarray(np.asarray(Wq, np.float32)[sl, :].T).astype(bf),
                "wk": np.ascontiguousarray(np.asarray(Wk, np.float32)[sl, :].T).astype(bf),
                "wv": np.ascontiguousarray(np.asarray(Wv, np.float32)[sl, :].T).astype(bf),
                "wo": np.ascontiguousarray(
                    wo_slice.reshape(GM // 128, 128, D).transpose(1, 0, 2).reshape(128, (GM // 128) * D)
                ).astype(bf),
                "bq": np.ascontiguousarray(np.asarray(bq, np.float32)[sl]),
                "bo": np.ascontiguousarray(bo_eff.astype(np.float32)),
            }
        )
    return in_maps


def expected_partial(c, x, Wq, bq, Wk, bk, Wv, bv, Wo, bo):
    """Numpy recomputation of core c's partial outT [D, S] (f32)."""
    b, g = c // 2, c % 2
    sl = slice(g * GM, (g + 1) * GM)
    xb = np.asarray(x, np.float32)[b]  # [S, D]
    Q = xb @ np.asarray(Wq, np.float32)[sl, :].T + np.asarray(bq, np.float32)[sl]
    K_ = xb @ np.asarray(Wk, np.float32)[sl, :].T + np.asarray(bk, np.float32)[sl]
    V = xb @ np.asarray(Wv, np.float32)[sl, :].T + np.asarray(bv, np.float32)[sl]
    out = np.empty((S, GM), np.float32)
    for h in range(GH):
        hs = slice(h * HD, (h + 1) * HD)
        sc = Q[:, hs] @ K_[:, hs].T / np.sqrt(HD)
        e = np.exp(sc - sc.max(-1, keepdims=True))
        out[:, hs] = (e @ V[:, hs]) / e.sum(-1, keepdims=True)
    y = out @ np.asarray(Wo, np.float32)[:, sl].T + np.asarray(bo, np.float32) / 2.0
    return np.ascontiguousarray(y.T)  # [D, S]


def kernel(x, Wq, bq, Wk, bk, Wv, bv, Wo, bo):
    in_maps = make_in_maps(x, Wq, bq, Wk, bk, Wv, bv, Wo, bo)
    global _last_in_maps
    _last_in_maps = in_maps
    nc = _get_program()
    res = run_bass_kernel_spmd(nc, in_maps, core_ids=list(range(8)))
    out = np.empty((B, S, D), np.float32)
    for b in range(B):
        acc = res.results[2 * b]["outT"].astype(np.float32) + res.results[
            2 * b + 1
        ]["outT"].astype(np.float32)
        out[b] = acc.T
    return out


# revision 93
# speedup vs baseline: 1.0692x; 1.0248x over previous
"""Multi-head self-attention (B=4, S=2048, D=1024, H=16, Hd=64) on 8 TRN2 cores.

Sharding: core c -> (batch b = c//2, head-group g = c%2 of 8 heads).
Each core computes its batch's 8 heads end-to-end plus the partial output
projection for its head group; the host sums the two head-group partials
per batch. No collectives.

Device layout is fully transposed: activations are [feature(partitions),
seq(free)]. QKV and output projections run in bf16. The attention part
(scores QK^T and PV) runs in fp8e4m3 with DoubleRow perf mode at 0.5
cycles/row. For scores, the two DoubleRow k-subtiles read the SAME 64-dim
Q/K data via a stride-0 middle dim (computing 2*QK^T); the doubling is
folded into the exp scale (1/16 instead of 1/8). V is stored as
[128 keys x kc-pair x 8 head x 65] with an all-ones 65th column producing
the softmax denominator inside the PV psum.

exp() splits across the Activation engine (exact exp, fp8 output) and the
DVE (Schraudolph: fp8e4m3 bits = trunc(s*scale*8/ln2 + 56.05) via one
tensor_scalar mult+add writing int8). Attention is pipelined as 16 jobs
(head, q-half) with PV lagging 4 jobs; Q/K projection m-tiles 1..3 are
emitted piecewise between score chunks so the PE fills exp-bound gaps.
K's bias is dropped (softmax-invariant); V's bias is folded into the
output-projection bias on the host.
"""

from contextlib import ExitStack

import numpy as np
import ml_dtypes

import concourse.bass as bass
import concourse.tile as tile
from concourse import mybir
from concourse.bass_utils import run_bass_kernel_spmd
from concourse.vector_clock import ScopedClock
from bass_rust import InstNoOp, SyncInfo

BF16 = mybir.dt.bfloat16
F32 = mybir.dt.float32
FP8 = mybir.dt.float8e4
I8 = mybir.dt.int8
AF = mybir.ActivationFunctionType
ALU = mybir.AluOpType
DR = mybir.MatmulPerfMode.DoubleRow

B, S, D = 4, 2048, 1024
H, HD = 16, 64
GH = 8          # heads per core (head-group size)
GM = GH * HD    # 512 head dims per core
NDC = 8         # d chunks of 128 (contraction for projections)
NKC = 16        # k chunks of 128
VW = HD + 1     # 65: per-head V columns + ones column
VWP = HD + 2    # 66: v8 stores heads 2-byte aligned (dual-fp8 LW requires
                # even weight byte offsets; 65-wide heads would give odd ones)
JW = S // 2     # 1024: q-width of one attention job
PV_LAG = 4      # jobs between scores(j) and PV(j)

# scores psum holds 2*QK^T (stride-0 DoubleRow pair), so exp scale is 1/16.
EXP_SCALE = 0.0625
SCH_MUL = EXP_SCALE * 8.0 / float(np.log(2.0))
SCH_ADD = 56.05


def _mk_pattern(counts):
    accs = {k: 0.0 for k in counts}
    total = sum(counts.values())
    out = []
    for _ in range(total):
        for k in counts:
            accs[k] += counts[k] / total
        k = max(accs, key=lambda q: accs[q])
        accs[k] -= 1.0
        out.append(k)
    return out


# exp engine per (kc) chunk within a job: 'A' = Act exact, 'D' = DVE
# Schraudolph. Early jobs carry QK copies on Act + V copies on DVE; later
# jobs carry norm work on DVE.
_PAT_EARLY = _mk_pattern({"A": 8, "D": 8})
_PAT_STEADY = _mk_pattern({"A": 11, "D": 5})
_PAT_STEADY2 = _mk_pattern({"A": 9, "D": 7})

_META_TYPES = ("TileBranchInst", "BassTileLoopBlock", "BassTilePoolBoundary")


class _TileCtx(tile.TileContext):
    """Splits multi-sem-wait instructions: the pinned walrus rejects any TPB
    instruction carrying more than one sem-wait, while Tile emits joins and a
    global end-of-context drain with several."""

    def _split_waits(self, ordered):
        nc = self.nc
        for bb_name, insts in ordered.items():
            out = []
            for inst in insts:
                si = inst.sync_info
                if (
                    si is not None
                    and si.on_wait
                    and len(si.on_wait) > 1
                    and type(inst).__name__ not in _META_TYPES
                    and inst.engine != mybir.EngineType.Unassigned
                ):
                    waits = list(si.on_wait)
                    for w in waits[:-1]:
                        nop = InstNoOp(
                            name=nc.get_next_instruction_name(), ins=[], outs=[]
                        )
                        nop.engine = inst.engine
                        nop.sync_info = SyncInfo(on_wait=[w], on_update=[])
                        out.append(nop)
                    inst.sync_info = SyncInfo(
                        on_wait=[waits[-1]], on_update=list(si.on_update)
                    )
                out.append(inst)
            ordered[bb_name] = out

    def _lower_ordered_insts(self, ordered):
        self._split_waits(ordered)
        super()._lower_ordered_insts(ordered)

    def _drain_and_barrier(self, tick_clock, wait_clock):
        drain_inst = self.nc.sync.drain()
        wait_clock.add_sem_waits(
            drain_inst.ins, ScopedClock({None: tick_clock.global_clock})
        )
        si = drain_inst.ins.sync_info
        waits = list(si.on_wait) if si is not None else []
        if len(waits) > 1:
            drain_inst.ins.sync_info = SyncInfo(
                on_wait=waits[:1], on_update=list(si.on_update)
            )
            for w in waits[1:]:
                extra = self.nc.sync.drain()
                extra.ins.sync_info = SyncInfo(on_wait=[w], on_update=[])

        self.nc.all_engine_barrier()
        assert self.sems is not None
        popped = self.nc._tile_sem_poison_stack.pop()
        assert popped is self._sem_poison
        self.nc.clear_and_free_semaphores(list(self.sems.allocated().values()))
        self.nc.all_engine_barrier()


def _build_program():
    nc = bass.Bass(trn_type="TRN2", debug=False, num_devices=8)

    xT = nc.dram_tensor("xT", [D, S], BF16, kind="ExternalInput").ap()
    wq = nc.dram_tensor("wq", [D, GM], BF16, kind="ExternalInput").ap()
    wk = nc.dram_tensor("wk", [D, GM], BF16, kind="ExternalInput").ap()
    wv = nc.dram_tensor("wv", [D, GM], BF16, kind="ExternalInput").ap()
    # pair-major-reordered Wo.T slice: [128, 4 pairs x 1024]
    wo = nc.dram_tensor("wo", [128, (GM // 128) * D], BF16, kind="ExternalInput").ap()
    bq = nc.dram_tensor("bq", [GM], F32, kind="ExternalInput").ap()
    bo = nc.dram_tensor("bo", [D], F32, kind="ExternalInput").ap()
    outT = nc.dram_tensor("outT", [D, S], BF16, kind="ExternalOutput").ap()

    with _TileCtx(nc) as tc, ExitStack() as ctx:
        const_pool = ctx.enter_context(tc.tile_pool(name="const", bufs=1))
        act_pool = ctx.enter_context(tc.tile_pool(name="acts", bufs=1))

        # ---- constants / weights / inputs -------------------------------
        bq_sb = const_pool.tile([128, GM // 128], F32, tag="bq")
        nc.sync.dma_start(bq_sb[:], bq.rearrange("(c p) -> p c", p=128))
        bo_sb = const_pool.tile([128, NDC], F32, tag="bo")
        nc.sync.dma_start(bo_sb[:], bo.rearrange("(c p) -> p c", p=128))
        wo_sb = const_pool.tile([128, (GM // 128) * D], BF16, tag="wo")
        nc.sync.dma_start(wo_sb[:], wo[:, :])

        # persistent activations. qt8/kt8: [128, slot t, S]; head h lives at
        # partitions [64*(h%2), +64) of slot h//2.
        qt8 = act_pool.tile([128, GH // 2, S], FP8, tag="qt8")
        kt8 = act_pool.tile([128, GH // 2, S], FP8, tag="kt8")
        v8 = act_pool.tile([128, NKC // 2, 2, GH, VWP], FP8, tag="v8")
        nc.gpsimd.memset(v8[:, :, :, :, HD:VWP], 1.0)
        otp = [
            act_pool.tile([128, S], BF16, name=f"otp{t}", tag=f"otp{t}")
            for t in range(GH // 2)
        ]

        phase1 = ExitStack()
        w_pool = phase1.enter_context(tc.tile_pool(name="wts", bufs=1))
        xt = w_pool.tile([128, NDC * S], BF16, tag="xt")
        for t in range(NDC):
            eng = nc.sync if t % 2 == 0 else nc.scalar
            eng.dma_start(
                xt[:, t * S : (t + 1) * S], xT[t * 128 : (t + 1) * 128, :]
            )
        wv_sb = w_pool.tile([128, NDC * GM], BF16, tag="wv")
        nc.scalar.dma_start(
            wv_sb[:].rearrange("p (c m) -> p c m", m=GM),
            wv.rearrange("(c p) m -> p c m", p=128),
        )
        wq_sb = w_pool.tile([128, NDC * GM], BF16, tag="wq")
        nc.sync.dma_start(
            wq_sb[:].rearrange("p (c m) -> p c m", m=GM),
            wq.rearrange("(c p) m -> p c m", p=128),
        )
        wk_sb = w_pool.tile([128, NDC * GM], BF16, tag="wk")
        nc.sync.dma_start(
            wk_sb[:].rearrange("p (c m) -> p c m", m=GM),
            wk.rearrange("(c p) m -> p c m", p=128),
        )

        # ---- attention + projections, software-pipelined ----------------
        with tc.tile_pool(name="s_psum", bufs=2, space="PSUM") as s_psum, \
             tc.tile_pool(name="slab", bufs=PV_LAG + 1) as slab_pool, \
             tc.tile_pool(name="norm", bufs=1) as norm_pool, \
             tc.tile_pool(name="bcast", bufs=2) as bc_pool, \
             tc.tile_pool(name="shift", bufs=1) as shift_pool, \
             tc.tile_pool(name="dscr", bufs=2, space="DRAM") as dram_pool:

            def qk_quarter(t, w_sb, dst, qq):
                """Generator: Q or K projection m-tile t, q-quarter qq (512).
                Yields after small matmul groups so callers can interleave."""
                ps = qk_psum.tile([128, 512], F32, tag="qkp")
                q0 = qq * 512
                for dc in range(NDC):
                    nc.tensor.matmul(
                        ps[:],
                        w_sb[:, dc * GM + t * 128 : dc * GM + (t + 1) * 128],
                        xt[:, dc * S + q0 : dc * S + q0 + 512],
                        start=(dc == 0),
                        stop=(dc == NDC - 1),
                    )
                    if dc % 2 == 1:
                        yield
                if dst is qt8:
                    nc.scalar.activation(
                        dst[:, t, q0 : q0 + 512], ps[:], AF.Identity,
                        bias=bq_sb[:, t : t + 1],
                    )
                else:
                    nc.scalar.activation(dst[:, t, q0 : q0 + 512], ps[:], AF.Copy)
                yield

            def v_chunk(si, v_psum):
                """Generator: V projection keys-chunk si -> v8 (fp8)."""
                ps = v_psum.tile([128, GM], F32, tag="vp")
                for dc in range(NDC):
                    nc.tensor.matmul(
                        ps[:],
                        xt[:, dc * S + si * 128 : dc * S + (si + 1) * 128],
                        wv_sb[:, dc * GM : (dc + 1) * GM],
                        start=(dc == 0),
                        stop=(dc == NDC - 1),
                    )
                    if dc % 4 == 3:
                        yield
                nc.vector.tensor_copy(
                    v8[:, si // 2, si % 2, :, 0:HD],
                    ps[:].rearrange("p (h d) -> p h d", h=GH),
                )
                yield

            # jobs: j = 2*h + qh
            slabs = [None] * (2 * GH)

            s_pools = [s_psum]
            chunk_ctr = [0]

            def issue_job_scores(j, fillers, pattern, inject=None):
                """Scores+exp for job j, pulling filler matmul groups from
                `fillers` (a list of active generators) between chunks.
                `inject` maps chunk index -> thunk issued at that point."""
                h, qh = j // 2, j % 2
                slot, p0 = h // 2, 64 * (h % 2)
                sl = slab_pool.tile([128, NKC // 2, 2, JW], FP8, tag="slab")
                slabs[j] = sl
                psl = slice(p0, p0 + 64)
                for kc in range(NKC):
                    if inject and kc in inject:
                        inject[kc]()
                    u, jj = kc // 2, kc % 2
                    # kt8/qt8 [64, 1, N] viewed as a stride-0 [64, 2, N]
                    lhsT = kt8[psl, slot, kc * 128 : (kc + 1) * 128] \
                        .unsqueeze(1).broadcast_to([64, 2, 128])
                    sp = s_pools[chunk_ctr[0] % len(s_pools)].tile(
                        [128, JW], F32, tag="sp"
                    )
                    chunk_ctr[0] += 1
                    for qq in range(JW // 512):
                        q0 = qh * JW + qq * 512
                        rhs = qt8[psl, slot, q0 : q0 + 512] \
                            .unsqueeze(1).broadcast_to([64, 2, 512])
                        nc.tensor.matmul(
                            sp[:, qq * 512 : (qq + 1) * 512],
                            lhsT,
                            rhs,
                            start=True,
                            stop=True,
                            perf_mode=DR,
                        )
                    dst = sl[:, u, jj, :]
                    if pattern[kc] == "A":
                        nc.scalar.activation(dst, sp[:], AF.Exp, scale=EXP_SCALE)
                    else:
                        nc.vector.tensor_scalar(
                            dst.bitcast(I8), sp[:],
                            SCH_MUL, SCH_ADD, ALU.mult, ALU.add,
                        )
                    # pull ~1.4 filler steps per chunk so the 12 QK halves
                    # (slots 1-3) finish by the end of job 4
                    fillers_budget[0] += 1.4
                    while fillers and fillers_budget[0] >= 1.0:
                        try:
                            next(fillers[0])
                            fillers_budget[0] -= 1.0
                        except StopIteration:
                            fillers.pop(0)

            pending = {}  # j -> (po tile, bc tile)

            def issue_pv_den(j, o_psum):
                """PV matmuls + reciprocal + the den-broadcast DMA roundtrip.
                The otp multiply is deferred (issue_norm_mult) so the DVE
                never stalls in-order on the broadcast DMA."""
                h = j // 2
                sl = slabs[j]
                po = o_psum.tile([128, JW], F32, tag="op")
                for u in range(NKC // 2):
                    lhsT = v8[:, u, :, h, 0:VW]
                    for qs in range(JW // 512):
                        nc.tensor.matmul(
                            po[0:VW, qs * 512 : (qs + 1) * 512],
                            lhsT,
                            sl[:, u, :, qs * 512 : (qs + 1) * 512],
                            start=(u == 0),
                            stop=(u == NKC // 2 - 1),
                            perf_mode=DR,
                        )
                nrm = norm_pool.tile([VW, JW], F32, tag="nrm")
                nc.vector.reciprocal(nrm[HD:VW, :], po[HD:VW, :])
                scr = dram_pool.tile([JW], F32, tag="scr")
                nc.sync.dma_start(scr.unsqueeze(0), nrm[HD:VW, :])
                bc = bc_pool.tile([HD, JW], F32, tag="bc")
                nc.sync.dma_start(bc[:], scr.unsqueeze(0).broadcast_to([HD, JW]))
                pending[j] = (po, bc)
                slabs[j] = None
                issue_norm_mult(j)

            def issue_norm_mult(j):
                if j not in pending:
                    return
                po, bc = pending.pop(j)
                h, qh = j // 2, j % 2
                t = h // 2
                qsl = slice(qh * JW, (qh + 1) * JW)
                if h % 2 == 0:
                    nc.vector.tensor_mul(otp[t][0:HD, qsl], po[0:HD, :], bc[:])
                else:
                    tmp = shift_pool.tile([HD, JW], BF16, tag="tmp")
                    nc.vector.tensor_mul(tmp[:], po[0:HD, :], bc[:])
                    nc.sync.dma_start(otp[t][HD:128, qsl], tmp[:])

            # -- prefix: QK slot 0, then V (PE-dense, exp engines idle) --
            qk_stack = ExitStack()
            qk_psum = qk_stack.enter_context(
                tc.tile_pool(name="qk_psum", bufs=2, space="PSUM")
            )
            for w_sb, dst in ((wq_sb, qt8), (wk_sb, kt8)):
                for qq in range(4):
                    for _ in qk_quarter(0, w_sb, dst, qq):
                        pass
            v_stack = ExitStack()
            v_psum = v_stack.enter_context(
                tc.tile_pool(name="v_psum", bufs=2, space="PSUM")
            )
            for si in range(NKC):
                for _ in v_chunk(si, v_psum):
                    pass
            v_stack.close()

            # -- pipelined jobs --
            # fillers: QK slots 1..3 emitted between score chunks of jobs 0..3
            fillers = []
            fillers_budget = [0.0]
            for t in range(1, GH // 2):
                for w_sb, dst in ((wq_sb, qt8), (wk_sb, kt8)):
                    for qq in range(4):
                        fillers.append(qk_quarter(t, w_sb, dst, qq))

            o_stack = ExitStack()
            o_psum = None
            for j in range(2 * GH):
                if j < 5:
                    pattern = _PAT_EARLY
                else:
                    pattern = _PAT_STEADY
                issue_job_scores(j, fillers, pattern)
                if j == PV_LAG - 1:
                    # drain remaining QK fillers; free their psum for PV
                    while fillers:
                        try:
                            next(fillers[0])
                        except StopIteration:
                            fillers.pop(0)
                    qk_stack.close()
                    o_psum = o_stack.enter_context(
                        tc.tile_pool(name="o_psum", bufs=1, space="PSUM")
                    )
                    s_x_stack = ExitStack()
                    s_x = s_x_stack.enter_context(
                        tc.tile_pool(name="s_x", bufs=1, space="PSUM")
                    )
                    s_pools.extend([s_psum, s_x])
                if j >= PV_LAG:
                    issue_pv_den(j - PV_LAG, o_psum)
            # trailing PVs: reuse s_x's banks as a second PV buffer so the
            # per-job den/mult chains overlap. The output projection runs
            # interleaved, borrowing idle scores-psum tiles: per (ec, qh)
            # half, the mt0-2 accumulation pre-runs during the trailing
            # chains and only the mt3 matmuls wait on the final norm.
            s_x_stack.close()
            o2_stack = ExitStack()
            o2 = o2_stack.enter_context(
                tc.tile_pool(name="o2_psum", bufs=1, space="PSUM")
            )
            y_pool = o2_stack.enter_context(tc.tile_pool(name="y", bufs=4))

            out_tiles = {}
            # finish-loop psum rotation: 2 scores bufs + the two freed
            # trailing-PV pools = depth 4, so mt0-2 prework runs ahead of
            # the critical mt3->copy chain
            out_pools = [s_psum, s_psum, o2, o_psum]

            def out_prework(i):
                ec, qh = i // 2, i % 2
                pool = out_pools[i % 4]
                ps = pool.tile([128, JW], F32, tag="sp" if pool is s_psum else "op")
                out_tiles[i] = ps
                for mt in range(GM // 128 - 1):
                    lhsT = wo_sb[:, mt * D + ec * 128 : mt * D + (ec + 1) * 128]
                    for qb in range(2):
                        q0 = qh * JW + qb * 512
                        nc.tensor.matmul(
                            ps[:, qb * 512 : (qb + 1) * 512],
                            lhsT,
                            otp[mt][:, q0 : q0 + 512],
                            start=(mt == 0),
                            stop=False,
                        )

            def out_finish(i):
                ec, qh = i // 2, i % 2
                ps = out_tiles.pop(i)
                mt = GM // 128 - 1
                lhsT = wo_sb[:, mt * D + ec * 128 : mt * D + (ec + 1) * 128]
                for qb in range(2):
                    q0 = qh * JW + qb * 512
                    nc.tensor.matmul(
                        ps[:, qb * 512 : (qb + 1) * 512],
                        lhsT,
                        otp[mt][:, q0 : q0 + 512],
                        start=False,
                        stop=True,
                    )
                y_sb = y_pool.tile([128, JW], BF16, tag="y")
                if i % 2 == 0:
                    nc.scalar.activation(
                        y_sb[:], ps[:], AF.Identity, bias=bo_sb[:, ec : ec + 1]
                    )
                else:
                    nc.vector.tensor_scalar_add(
                        y_sb[:], ps[:], bo_sb[:, ec : ec + 1]
                    )
                eng = nc.sync if i % 2 == 0 else nc.scalar
                eng.dma_start(
                    outT[ec * 128 : (ec + 1) * 128, qh * JW : (qh + 1) * JW],
                    y_sb[:],
                )

            for i, j in enumerate(range(2 * GH - PV_LAG, 2 * GH)):
                issue_pv_den(j, o2 if i % 2 == 0 else o_psum)
                issue_norm_mult(j - 1)
                if i == 2:
                    out_prework(0)
                if i == 3:
                    out_prework(1)
            issue_norm_mult(2 * GH - 1)
            out_prework(2)
            out_prework(3)
            for i in range(2 * NDC):
                out_finish(i)
                if i + 4 < 2 * NDC:
                    out_prework(i + 4)
            o2_stack.close()
            o_stack.close()
        phase1.close()

    return nc


_NC = None
_last_in_maps = None


def _get_program():
    global _NC
    if _NC is None:
        _NC = _build_program()
    return _NC


def make_in_maps(x, Wq, bq, Wk, bk, Wv, bv, Wo, bo):
    x = np.asarray(x, np.float32)
    bf = ml_dtypes.bfloat16
    in_maps = []
    for c in range(8):
        b, g = c // 2, c % 2
        sl = slice(g * GM, (g + 1) * GM)
        wo_slice = np.asarray(Wo, np.float32)[:, sl].T  # [512, 1024]
        # fold bv and half of bo into the output bias
        bo_eff = np.asarray(bo, np.float32) / 2.0 + np.asarray(bv, np.float32)[sl] @ wo_slice
        in_maps.append(
            {
                "xT": np.ascontiguousarray(x[b].T).astype(bf),
                "wq": np.ascontiguousarray(np.asarray(Wq, np.float32)[sl, :].T).astype(bf),
                "wk": np.ascontiguousarray(np.asarray(Wk, np.float32)[sl, :].T).astype(bf),
                "wv": np.ascontiguousarray(np.asarray(Wv, np.float32)[sl, :].T).astype(bf),
                "wo": np.ascontiguousarray(
                    wo_slice.reshape(GM // 128, 128, D).transpose(1, 0, 2).reshape(128, (GM // 128) * D)
                ).astype(bf),
                "bq": np.ascontiguousarray(np.asarray(bq, np.float32)[sl]),
                "bo": np.ascontiguousarray(bo_eff.astype(np.float32)),
            }
        )
    return in_maps


def expected_partial(c, x, Wq, bq, Wk, bk, Wv, bv, Wo, bo):
    """Numpy recomputation of core c's partial outT [D, S] (f32)."""
    b, g = c // 2, c % 2
    sl = slice(g * GM, (g + 1) * GM)
    xb = np.asarray(x, np.float32)[b]  # [S, D]
    Q = xb @ np.asarray(Wq, np.float32)[sl, :].T + np.asarray(bq, np.float32)[sl]
    K_ = xb @ np.asarray(Wk, np.float32)[sl, :].T + np.asarray(bk, np.float32)[sl]
    V = xb @ np.asarray(Wv, np.float32)[sl, :].T + np.asarray(bv, np.float32)[sl]
    out = np.empty((S, GM), np.float32)
    for h in range(GH):
        hs = slice(h * HD, (h + 1) * HD)
        sc = Q[:, hs] @ K_[:, hs].T / np.sqrt(HD)
        e = np.exp(sc - sc.max(-1, keepdims=True))
        out[:, hs] = (e @ V[:, hs]) / e.sum(-1, keepdims=True)
    y = out @ np.asarray(Wo, np.float32)[:, sl].T + np.asarray(bo, np.float32) / 2.0
    return np.ascontiguousarray(y.T)  # [D, S]


def kernel(x, Wq, bq, Wk, bk, Wv, bv, Wo, bo):
    in_maps = make_in_maps(x, Wq, bq, Wk, bk, Wv, bv, Wo, bo)
    global _last_in_maps
    _last_in_maps = in_maps
    nc = _get_program()
    res = run_bass_kernel_spmd(nc, in_maps, core_ids=list(range(8)))
    out = np.empty((B, S, D), np.float32)
    for b in range(B):
        acc = res.results[2 * b]["outT"].astype(np.float32) + res.results[
            2 * b + 1
        ]["outT"].astype(np.float32)
        out[b] = acc.T
    return out
